# revision 1
# baseline (speedup 1.0000x reference)
"""PointsFusion Trainium2 kernel.

Pipeline per batch b (B=4, N=4096, k=32):
  knn1 = 32-NN of p1 in p1, knn2 = 32-NN of p1 in p2 (exact, via DVE 8-max rounds)
  gather neighbor coords, features (resi, dist) -> conv(4->64)->BN->relu
  -> conv(64->64)->BN->relu -> conv(64->128)->BN->relu -> channel-max scores
  -> softmax over 64 neighbors -> weighted sum of neighbor coords.

Sharding: 8 cores = (batch b, half h of the 4096 query points). BatchNorm uses
global batch stats -> 3 tiny AllReduces of per-channel sum/sumsq.

Layouts (per 128-query tile):
  pixel space: 16 chunks of 512; chunk c = kn*8+g, pixel j = c*512 + s*16 + p
  (g = query group, p = query-in-group, s = neighbor slot, kn = which knn).
  64-channel activations are packed [128, 4096]: chunk c lives at partitions
  64*(c%2)..+64, free 512*(c//2)..+512 (keeps matmul rhs bases in {0, 64}).

Self-contained: hardcodes shapes; no sibling imports.
"""

import sys

import numpy as np

for _p in ("/opt/trn_rl_repo", "/opt/pypackages"):
    if _p not in sys.path:
        sys.path.append(_p)

import concourse.bass as bass  # noqa: E402  (imported for side effects/typing)
import concourse.mybir as mybir  # noqa: E402
import concourse.tile as tile  # noqa: E402
from concourse import bacc, bass_isa  # noqa: E402
from concourse.bass_utils import run_bass_kernel_spmd  # noqa: E402
from concourse.masks import make_identity  # noqa: E402

F32 = mybir.dt.float32
F32R = mybir.dt.float32r
U16 = mybir.dt.uint16
I16 = mybir.dt.int16
AF = mybir.ActivationFunctionType
OP = mybir.AluOpType

NCORES = 8
B = 4
N = 4096          # candidate points per batch
KNN = 32          # neighbors per knn
QPC = 2048        # query points per core
NT = 16           # query tiles of 128 per core
C1, C2, C3 = 64, 64, 128
NTOT = float(B * N * 2 * KNN)   # BN stat count (global)
BN_EPS = 1e-3
NEG = -1.0e30


def _pk(cc):
    """packed [128, 4096] slice coords for chunk cc."""
    return 64 * (cc % 2), 512 * (cc // 2)


def _build_program(single=False):
    nc = bacc.Bacc(
        "TRN2", target_bir_lowering=False, debug=False,
        num_devices=1 if single else NCORES,
    )
    nc._single_core_nocoll = single

    ap = {}
    def din(name, shape):
        ap[name] = nc.dram_tensor(name, shape, F32, kind="ExternalInput").ap()
    din("qf", [4, QPC])
    din("t1", [4, N])
    din("t2", [4, N])
    din("gt", [128, N])
    din("qr", [4, QPC])
    din("qsq", [128, NT])
    din("w1t", [4, C1])
    din("w2t", [128, C2])     # duplicated at partition 64
    din("w3t", [128, C3])     # duplicated at partition 64
    din("gt2", [128, N])
    din("gb1", [C1, 2])
    din("gb2", [C2, 2])
    din("gb3", [C3, 2])
    din("selw", [8, 128])

    ap["out"] = nc.dram_tensor("out", [3, QPC], F32, kind="ExternalOutput").ap()

    ap["y1d"] = nc.dram_tensor("y1d", [NT, 128, 4096], F32).ap()
    ap["y2d"] = nc.dram_tensor("y2d", [NT, 128, 4096], F32).ap()
    ap["y3d"] = nc.dram_tensor("y3d", [NT, C3, 8192], F32).ap()
    ap["g1d"] = nc.dram_tensor("g1d", [NT, 128, 512], F32).ap()
    ap["g2d"] = nc.dram_tensor("g2d", [NT, 128, 512], F32).ap()
    ap["dsd"] = nc.dram_tensor("dsd", [NT, 8192], F32).ap()
    for i, c in ((0, C1), (1, C2), (2, C3)):
        ap[f"arin{i}"] = nc.dram_tensor(f"arin{i}", [c * 2], F32).ap()
        ap[f"arout{i}"] = nc.dram_tensor(f"arout{i}", [c * 2], F32).ap()

    with tile.TileContext(nc) as tc:
        _kernel_body(tc, ap)
    nc.compile()
    return nc


def _kernel_body(tc, d):
    nc = tc.nc
    from contextlib import ExitStack

    ctx = ExitStack()
    with ctx:
        cpool = ctx.enter_context(tc.tile_pool(name="consts", bufs=1))
        t1 = cpool.tile([4, N], F32)
        t2 = cpool.tile([4, N], F32)
        gt = cpool.tile([128, N], F32)
        qf = cpool.tile([4, QPC], F32)
        qr = cpool.tile([4, QPC], F32)
        qsq = cpool.tile([128, NT], F32)
        w1 = cpool.tile([4, C1], F32)
        w2 = cpool.tile([128, C2], F32)
        w3 = cpool.tile([128, C3], F32)
        gb1 = cpool.tile([C1, 2], F32)
        gb2 = cpool.tile([C2, 2], F32)
        gb3 = cpool.tile([C3, 2], F32)
        gt2 = cpool.tile([128, N], F32)
        selw = cpool.tile([8, 128], F32)
        ident = cpool.tile([128, 128], F32)
        make_identity(nc, ident[:])
        for nm, sb in [("t1", t1), ("t2", t2), ("gt", gt), ("gt2", gt2),
                       ("qf", qf),
                       ("qr", qr), ("qsq", qsq), ("w1t", w1), ("w2t", w2),
                       ("w3t", w3), ("gb1", gb1), ("gb2", gb2), ("gb3", gb3),
                       ("selw", selw)]:
            nc.sync.dma_start(out=sb[:], in_=d[nm][:])

        spool = ctx.enter_context(tc.tile_pool(name="stats", bufs=1))
        sm1 = spool.tile([C1, NT * 16], F32)
        sq1 = spool.tile([C1, NT * 16], F32)
        sm2 = spool.tile([C2, NT * 16], F32)
        sq2 = spool.tile([C2, NT * 16], F32)
        sm3 = spool.tile([C3, NT * 16], F32)
        sq3 = spool.tile([C3, NT * 16], F32)
        ab1 = spool.tile([128, 2], F32)   # col0 = scale a, col1 = bias b (dup at 64)
        ab2 = spool.tile([128, 2], F32)
        ab3 = spool.tile([C3, 2], F32)

        # ---------------- Phase 1: knn + gather + feat + conv1 ----------------
        with tc.tile_pool(name="p1m", bufs=2) as mpool, \
             tc.tile_pool(name="p1psum", bufs=2, space="PSUM") as pp, \
             tc.tile_pool(name="p1tp", bufs=2, space="PSUM") as tpp, \
             tc.tile_pool(name="p1cpsum", bufs=3, space="PSUM") as cp, \
             tc.tile_pool(name="p1feat", bufs=1) as fpool, \
             tc.tile_pool(name="p1work", bufs=2) as wp, \
             tc.tile_pool(name="p1y", bufs=2) as yp:
            for t in range(NT):
                msb = mpool.tile([128, N], F32, tag="msb")
                vals = wp.tile([128, 64], F32, tag="vals")
                idxu = wp.tile([128, 64], U16, tag="idxu")
                idxi = wp.tile([128, 64], I16, tag="idxi")
                for kn, tab in ((0, t1), (1, t2)):
                    # M = 2 q.c - |c|^2 (maximize == nearest)
                    for ch in range(8):
                        pm = pp.tile([128, 512], F32, tag="pm")
                        nc.tensor.matmul(
                            out=pm[:],
                            lhsT=qf[:, t * 128:(t + 1) * 128],
                            rhs=tab[:, ch * 512:(ch + 1) * 512],
                            start=True, stop=True,
                        )
                        nc.scalar.activation(
                            out=msb[:, ch * 512:(ch + 1) * 512], in_=pm[:],
                            func=AF.Identity)
                    # top-32 rounds
                    for r in range(4):
                        v8 = vals[:, kn * 32 + r * 8: kn * 32 + r * 8 + 8]
                        i8 = idxu[:, kn * 32 + r * 8: kn * 32 + r * 8 + 8]
                        nc.vector.max(out=v8, in_=msb[:])
                        nc.vector.max_index(out=i8, in_max=v8, in_values=msb[:])
                        if r < 3:
                            nc.vector.match_replace(
                                out=msb[:], in_to_replace=v8,
                                in_values=msb[:], imm_value=NEG)
                nc.vector.tensor_copy(out=idxi[:], in_=idxu[:])

                # gather neighbor coords; both tables carry xyz on band rows
                # 16g+{0..2} (gt = p1 for knn1, gt2 = p2 for knn2); spill raw
                # for the fusion phase
                g1 = wp.tile([128, 512], F32, tag="g1")
                g2 = wp.tile([128, 512], F32, tag="g2")
                nc.gpsimd.ap_gather(
                    out_ap=g1[:], in_ap=gt[:], idxs_ap=idxi[:, 0:32],
                    channels=128, num_elems=N, d=1, num_idxs=512)
                nc.gpsimd.ap_gather(
                    out_ap=g2[:], in_ap=gt2[:], idxs_ap=idxi[:, 32:64],
                    channels=128, num_elems=N, d=1, num_idxs=512)
                nc.sync.dma_start(out=d["g1d"][t], in_=g1[:])
                nc.sync.dma_start(out=d["g2d"][t], in_=g2[:])

                # conv1 rhs must start at partition 0: DMA bands into a flat
                # [4, 8192] tile
                feat = fpool.tile([4, 8192], F32, tag="feat")
                for g in range(8):
                    nc.sync.dma_start(
                        out=feat[0:3, g * 512:(g + 1) * 512],
                        in_=g1[16 * g: 16 * g + 3, :])
                    nc.sync.dma_start(
                        out=feat[0:3, (8 + g) * 512:(9 + g) * 512],
                        in_=g2[16 * g: 16 * g + 3, :])

                # dist = sqrt(max(|q|^2 - val, 0)) into feat row 3
                d2 = wp.tile([128, 64], F32, tag="d2")
                nc.vector.tensor_scalar(
                    out=d2[:], in0=vals[:], scalar1=qsq[:, t:t + 1],
                    scalar2=-1.0, op0=OP.subtract, op1=OP.mult)
                nc.vector.tensor_scalar_max(d2[:], d2[:], 0.0)
                nc.scalar.activation(out=d2[:], in_=d2[:], func=AF.Sqrt)
                # shuffle dist to pixel layout: PE-transpose to [nbr, query],
                # then per-chunk DMAs with contiguous 16-wide runs
                dtp = tpp.tile([64, 128], F32, tag="dtp")
                nc.tensor.transpose(out=dtp[:], in_=d2[:], identity=ident[:])
                d2t = wp.tile([64, 128], F32, tag="d2t")
                nc.scalar.activation(out=d2t[:], in_=dtp[:], func=AF.Identity)
                for kn in (0, 1):
                    for g in range(8):
                        c = kn * 8 + g
                        nc.sync.dma_start(
                            out=feat[3:4, c * 512:(c + 1) * 512]
                                .rearrange("c (s p) -> c s p", s=32),
                            in_=d2t[kn * 32:(kn + 1) * 32,
                                    16 * g:16 * g + 16])

                # resi = nn - q (in place on coord rows)
                qrt = qr[0:3, t * 128:(t + 1) * 128]
                for kn in (0, 1):
                    nc.vector.tensor_tensor(
                        out=feat[0:3, kn * 4096:(kn + 1) * 4096]
                            .rearrange("c (g s p) -> c g s p", g=8, s=32),
                        in0=feat[0:3, kn * 4096:(kn + 1) * 4096]
                            .rearrange("c (g s p) -> c g s p", g=8, s=32),
                        in1=qrt.rearrange("c (g p) -> c g p", g=8)
                            .unsqueeze(2).to_broadcast([3, 8, 32, 16]),
                        op=OP.subtract)

                # conv1: 16 chunks -> y1 packed [128, 4096]
                y1 = yp.tile([128, 4096], F32, tag="y1")
                for c in range(16):
                    bp_, fo = _pk(c)
                    pc = cp.tile([C1, 512], F32, tag="pc1")
                    nc.tensor.matmul(
                        out=pc[:],
                        lhsT=w1[:],
                        rhs=feat[:, c * 512:(c + 1) * 512],
                        start=True, stop=True)
                    nc.scalar.activation(
                        out=y1[bp_:bp_ + 64, fo:fo + 512], in_=pc[:],
                        func=AF.Identity,
                        accum_out=sm1[:, t * 16 + c: t * 16 + c + 1])
                    sqs = wp.tile([C1, 512], F32, tag="sqs")
                    nc.scalar.activation(
                        out=sqs[:], in_=pc[:], func=AF.Square,
                        accum_out=sq1[:, t * 16 + c: t * 16 + c + 1])
                nc.sync.dma_start(out=d["y1d"][t], in_=y1[:])

        _bn_allreduce(tc, 0, sm1, sq1, gb1, ab1, d["arin0"], d["arout0"], True)

        # ---------------- Phase 2: apply BN1+relu, conv2 ----------------
        with tc.tile_pool(name="p2y", bufs=2) as yp, \
             tc.tile_pool(name="p2psum", bufs=4, space="PSUM") as cp, \
             tc.tile_pool(name="p2work", bufs=2) as wp:
            for t in range(NT):
                y1 = yp.tile([128, 4096], F32, tag="y1l")
                nc.sync.dma_start(out=y1[:], in_=d["y1d"][t])
                nc.scalar.activation(
                    out=y1[:], in_=y1[:], func=AF.Relu,
                    scale=ab1[:, 0:1], bias=ab1[:, 1:2])
                y2 = yp.tile([128, 4096], F32, tag="y2")
                for c in range(16):
                    bp_, fo = _pk(c)
                    pc = cp.tile([C2, 512], F32, tag="pc2")
                    nc.tensor.matmul(
                        out=pc[:], lhsT=w2[bp_:bp_ + 64, :],
                        rhs=y1[bp_:bp_ + 64, fo:fo + 512],
                        start=True, stop=True)
                    nc.scalar.activation(
                        out=y2[bp_:bp_ + 64, fo:fo + 512], in_=pc[:],
                        func=AF.Identity,
                        accum_out=sm2[:, t * 16 + c: t * 16 + c + 1])
                    sqs = wp.tile([C2, 512], F32, tag="sqs2")
                    nc.scalar.activation(
                        out=sqs[:], in_=pc[:], func=AF.Square,
                        accum_out=sq2[:, t * 16 + c: t * 16 + c + 1])
                nc.sync.dma_start(out=d["y2d"][t], in_=y2[:])

        _bn_allreduce(tc, 1, sm2, sq2, gb2, ab2, d["arin1"], d["arout1"], True)

        # ---------------- Phase 3: apply BN2+relu, conv3 ----------------
        with tc.tile_pool(name="p3y", bufs=2) as yp, \
             tc.tile_pool(name="p3psum", bufs=4, space="PSUM") as cp, \
             tc.tile_pool(name="p3work", bufs=2) as wp:
            for t in range(NT):
                y2 = yp.tile([128, 4096], F32, tag="y2l")
                nc.sync.dma_start(out=y2[:], in_=d["y2d"][t])
                nc.scalar.activation(
                    out=y2[:], in_=y2[:], func=AF.Relu,
                    scale=ab2[:, 0:1], bias=ab2[:, 1:2])
                y3 = yp.tile([C3, 8192], F32, tag="y3")
                for c in range(16):
                    bp_, fo = _pk(c)
                    pc = cp.tile([C3, 512], F32, tag="pc3")
                    nc.tensor.matmul(
                        out=pc[:], lhsT=w3[bp_:bp_ + 64, :],
                        rhs=y2[bp_:bp_ + 64, fo:fo + 512],
                        start=True, stop=True)
                    nc.scalar.activation(
                        out=y3[:, c * 512:(c + 1) * 512], in_=pc[:],
                        func=AF.Identity,
                        accum_out=sm3[:, t * 16 + c: t * 16 + c + 1])
                    sqs = wp.tile([C3, 512], F32, tag="sqs3")
                    nc.scalar.activation(
                        out=sqs[:], in_=pc[:], func=AF.Square,
                        accum_out=sq3[:, t * 16 + c: t * 16 + c + 1])
                nc.sync.dma_start(out=d["y3d"][t], in_=y3[:])

        _bn_allreduce(tc, 2, sm3, sq3, gb3, ab3, d["arin2"], d["arout2"], False)

        # ------------- Phase 4: scores, softmax, fusion, output -------------
        with tc.tile_pool(name="p4y", bufs=2) as yp, \
             tc.tile_pool(name="p4work", bufs=2) as wp, \
             tc.tile_pool(name="p4psum", bufs=2, space="PSUM") as pp4, \
             tc.tile_pool(name="p4out", bufs=1) as op_:
            outsb = op_.tile([4, QPC], F32)
            for t in range(NT):
                y3 = yp.tile([C3, 8192], F32, tag="y3l")
                nc.sync.dma_start(out=y3[:], in_=d["y3d"][t])
                nc.scalar.activation(
                    out=y3[:], in_=y3[:], func=AF.Relu,
                    scale=ab3[:, 0:1], bias=ab3[:, 1:2])
                # channel-max scores, split by knn half (engine partition
                # bases must be 32-aligned, so rows land via DMA)
                scA = wp.tile([8, 512], F32, tag="scA")
                scB = wp.tile([8, 512], F32, tag="scB")
                par = wp.tile([128, 512], F32, tag="par")
                for c in range(16):
                    nc.gpsimd.partition_all_reduce(
                        out_ap=par[:], in_ap=y3[:, c * 512:(c + 1) * 512],
                        channels=128, reduce_op=bass_isa.ReduceOp.max)
                    dst = scA if c < 8 else scB
                    nc.sync.dma_start(out=dst[c % 8: c % 8 + 1, :],
                                      in_=par[0:1, :])
                # softmax over the 64 neighbors of each query
                qmA = wp.tile([8, 16], F32, tag="qmA")
                qmB = wp.tile([8, 16], F32, tag="qmB")
                for sct, qm in ((scA, qmA), (scB, qmB)):
                    nc.vector.tensor_reduce(
                        out=qm[:],
                        in_=sct[:].rearrange("c (s p) -> c p s", s=32),
                        axis=mybir.AxisListType.X, op=OP.max)
                nc.vector.tensor_tensor(
                    out=qmA[:], in0=qmA[:], in1=qmB[:], op=OP.max)
                exA = wp.tile([8, 512], F32, tag="exA")
                exB = wp.tile([8, 512], F32, tag="exB")
                for sct, ext in ((scA, exA), (scB, exB)):
                    nc.vector.tensor_tensor(
                        out=ext[:].rearrange("c (s p) -> c s p", s=32),
                        in0=sct[:].rearrange("c (s p) -> c s p", s=32),
                        in1=qmA[:].unsqueeze(1).to_broadcast([8, 32, 16]),
                        op=OP.subtract)
                    nc.scalar.activation(out=ext[:], in_=ext[:], func=AF.Exp)
                esA = wp.tile([8, 16], F32, tag="esA")
                esB = wp.tile([8, 16], F32, tag="esB")
                for ext, est in ((exA, esA), (exB, esB)):
                    nc.vector.tensor_reduce(
                        out=est[:],
                        in_=ext[:].rearrange("c (s p) -> c p s", s=32),
                        axis=mybir.AxisListType.X, op=OP.add)
                nc.vector.tensor_tensor(
                    out=esA[:], in0=esA[:], in1=esB[:], op=OP.add)
                nc.vector.reciprocal(out=esA[:], in_=esA[:])
                for ext in (exA, exB):
                    nc.vector.tensor_tensor(
                        out=ext[:].rearrange("c (s p) -> c s p", s=32),
                        in0=ext[:].rearrange("c (s p) -> c s p", s=32),
                        in1=esA[:].unsqueeze(1).to_broadcast([8, 32, 16]),
                        op=OP.mult)
                # fusion: replicate weight rows onto band partitions via a
                # selector matmul, multiply with raw coords, segment-reduce
                g1 = wp.tile([128, 512], F32, tag="g1l")
                g2 = wp.tile([128, 512], F32, tag="g2l")
                nc.sync.dma_start(out=g1[:], in_=d["g1d"][t])
                nc.sync.dma_start(out=g2[:], in_=d["g2d"][t])
                wr1 = wp.tile([128, 512], F32, tag="wr1")
                wr2 = wp.tile([128, 512], F32, tag="wr2")
                for ext, wr in ((exA, wr1), (exB, wr2)):
                    pw = pp4.tile([128, 512], F32, tag="pw")
                    nc.tensor.matmul(
                        out=pw[:], lhsT=selw[:],
                        rhs=ext[:], start=True, stop=True)
                    nc.scalar.activation(out=wr[:], in_=pw[:], func=AF.Identity)
                pr = wp.tile([128, 512], F32, tag="pr")
                nc.vector.tensor_tensor(out=pr[:], in0=g1[:], in1=wr1[:],
                                        op=OP.mult)
                nc.vector.tensor_tensor(out=wr2[:], in0=g2[:], in1=wr2[:],
                                        op=OP.mult)
                nc.vector.tensor_tensor(out=pr[:], in0=pr[:], in1=wr2[:],
                                        op=OP.add)
                fp = wp.tile([128, 16], F32, tag="fp")
                nc.vector.tensor_reduce(
                    out=fp[:], in_=pr[:].rearrange("c (s p) -> c p s", s=32),
                    axis=mybir.AxisListType.X, op=OP.add)
                for g in range(8):
                    nc.sync.dma_start(
                        out=outsb[0:3,
                                  t * 128 + 16 * g: t * 128 + 16 * g + 16],
                        in_=fp[16 * g: 16 * g + 3, :])
            nc.sync.dma_start(out=d["out"][:], in_=outsb[0:3, :])


def _bn_allreduce(tc, li, sm, sq, gbe, ab, arin, arout, dup):
    """Reduce per-chunk stat slots, AllReduce across 8 cores, compute
    per-channel scale a = g*rsqrt(var+eps) and bias b = be - a*mean."""
    nc = tc.nc
    C = sm.shape[0]
    with tc.tile_pool(name=f"bn{li}", bufs=1) as bp:
        st = bp.tile([C, 2], F32)
        nc.vector.tensor_reduce(out=st[:, 0:1], in_=sm[:],
                                axis=mybir.AxisListType.X, op=OP.add)
        nc.vector.tensor_reduce(out=st[:, 1:2], in_=sq[:],
                                axis=mybir.AxisListType.X, op=OP.add)
        nc.sync.dma_start(out=arin[:], in_=st[:])
        if getattr(nc, "_single_core_nocoll", False):
            nc.sync.dma_start(out=arout[:], in_=arin[:])
        else:
            nc.gpsimd.collective_compute(
                "AllReduce", OP.add, replica_groups=[list(range(NCORES))],
                ins=[arin.opt()], outs=[arout.opt()])
        ar = bp.tile([C, 2], F32)
        nc.sync.dma_start(out=ar[:], in_=arout[:])
        mean = bp.tile([C, 1], F32)
        var = bp.tile([C, 1], F32)
        nc.vector.tensor_scalar_mul(mean[:], ar[:, 0:1], 1.0 / NTOT)
        nc.vector.tensor_scalar_mul(var[:], ar[:, 1:2], 1.0 / NTOT)
        m2 = bp.tile([C, 1], F32)
        nc.vector.tensor_tensor(out=m2[:], in0=mean[:], in1=mean[:], op=OP.mult)
        nc.vector.tensor_tensor(out=var[:], in0=var[:], in1=m2[:], op=OP.subtract)
        nc.vector.tensor_scalar_add(var[:], var[:], BN_EPS)
        nc.scalar.activation(out=var[:], in_=var[:], func=AF.Sqrt)
        nc.vector.reciprocal(out=var[:], in_=var[:])  # rsqrt(var+eps)
        nc.vector.tensor_tensor(out=ab[0:C, 0:1], in0=var[:], in1=gbe[:, 0:1],
                                op=OP.mult)            # a
        nc.vector.tensor_tensor(out=m2[:], in0=ab[0:C, 0:1], in1=mean[:],
                                op=OP.mult)
        nc.vector.tensor_tensor(out=ab[0:C, 1:2], in0=gbe[:, 1:2], in1=m2[:],
                                op=OP.subtract)        # b = be - a*mean
        if dup:
            nc.vector.tensor_copy(out=ab[C:2 * C, :], in_=ab[0:C, :])


_PROGRAM = None
LAST_RESULT = None


def _get_program():
    global _PROGRAM
    if _PROGRAM is None:
        _PROGRAM = _build_program()
    return _PROGRAM


def _prep_core_inputs(points1, points2, W1, W2, W3, gs, bes, b, h):
    p1 = points1[b]          # [3, N]
    p2 = points2[b]
    q = p1[:, h * QPC:(h + 1) * QPC]            # [3, QPC]
    qf = np.concatenate([2.0 * q, np.ones((1, QPC), np.float32)], axis=0)

    def cand_tab(p):
        sq = (p * p).sum(axis=0, keepdims=True)
        return np.concatenate([p, -sq], axis=0).astype(np.float32)  # [4, N]

    gtab = np.zeros((128, N), np.float32)
    gtab2 = np.zeros((128, N), np.float32)
    for g in range(8):
        gtab[16 * g + 0:16 * g + 3] = p1
        gtab2[16 * g + 0:16 * g + 3] = p2
    qraw = np.zeros((4, QPC), np.float32)
    qraw[0:3] = q
    qsqv = (q * q).sum(axis=0).reshape(NT, 128).T.astype(np.float32)  # [128, NT]

    def dup128(w):      # [64, C] -> [128, C] duplicated
        return np.concatenate([w, w], axis=0).astype(np.float32)

    selw = np.zeros((8, 128), np.float32)
    for g in range(8):
        for c3 in range(3):
            selw[g, 16 * g + c3] = 1.0

    return {
        "selw": selw,
        "qf": qf.astype(np.float32),
        "t1": cand_tab(p1), "t2": cand_tab(p2), "gt": gtab, "gt2": gtab2,
        "qr": qraw, "qsq": np.ascontiguousarray(qsqv),
        "w1t": np.ascontiguousarray(W1.T).astype(np.float32),
        "w2t": dup128(np.ascontiguousarray(W2.T)),
        "w3t": dup128(np.ascontiguousarray(W3.T)),
        "gb1": np.stack([gs[0], bes[0]], axis=1).astype(np.float32),
        "gb2": np.stack([gs[1], bes[1]], axis=1).astype(np.float32),
        "gb3": np.stack([gs[2], bes[2]], axis=1).astype(np.float32),
    }


def kernel(points1, points2, k, t, W1, b1, g1, be1, W2, b2, g2, be2,
           W3, b3, g3, be3):
    # b1/b2/b3 cancel inside train-mode BatchNorm; t is unused by the net.
    assert int(np.asarray(k)) == KNN
    points1 = np.asarray(points1, np.float32)
    points2 = np.asarray(points2, np.float32)
    gs = [np.asarray(g1, np.float32), np.asarray(g2, np.float32),
          np.asarray(g3, np.float32)]
    bes = [np.asarray(be1, np.float32), np.asarray(be2, np.float32),
           np.asarray(be3, np.float32)]
    Ws = [np.asarray(W1, np.float32), np.asarray(W2, np.float32),
          np.asarray(W3, np.float32)]

    in_maps = []
    for c in range(NCORES):
        b, h = divmod(c, 2)
        in_maps.append(_prep_core_inputs(points1, points2, *Ws, gs, bes, b, h))

    nc = _get_program()
    bkr = run_bass_kernel_spmd(nc, in_maps, list(range(NCORES)))
    global LAST_RESULT
    LAST_RESULT = bkr
    res = bkr.results

    out = np.zeros((B, 3, N), np.float32)
    for c in range(NCORES):
        b, h = divmod(c, 2)
        out[b, :, h * QPC:(h + 1) * QPC] = res[c]["out"]
    return out



# revision 12
# speedup vs baseline: 1.2408x; 1.2408x over previous
"""PointsFusion Trainium2 kernel.

Pipeline per batch b (B=4, N=4096, k=32):
  knn1 = 32-NN of p1 in p1, knn2 = 32-NN of p1 in p2 (exact, via DVE 8-max rounds)
  gather neighbor coords, features (resi, dist) -> conv(4->64)->BN->relu
  -> conv(64->64)->BN->relu -> conv(64->128)->BN->relu -> channel-max scores
  -> softmax over 64 neighbors -> weighted sum of neighbor coords.

Sharding: 8 cores = (batch b, half h of the 4096 query points). BatchNorm uses
global batch stats -> 3 tiny AllReduces of per-channel sum/sumsq.

Layouts (per 128-query tile):
  pixel space: 16 chunks of 512; chunk c = kn*8+g, pixel j = c*512 + s*16 + p
  (g = query group, p = query-in-group, s = neighbor slot, kn = which knn).
  64-channel activations are packed [128, 4096]: chunk c lives at partitions
  64*(c%2)..+64, free 512*(c//2)..+512 (keeps matmul rhs bases in {0, 64}).

Self-contained: hardcodes shapes; no sibling imports.
"""

import sys

import numpy as np

for _p in ("/opt/trn_rl_repo", "/opt/pypackages"):
    if _p not in sys.path:
        sys.path.append(_p)

import concourse.bass as bass  # noqa: E402  (imported for side effects/typing)
import concourse.mybir as mybir  # noqa: E402
import concourse.tile as tile  # noqa: E402
from concourse import bacc, bass_isa  # noqa: E402
from concourse.bass_utils import run_bass_kernel_spmd  # noqa: E402
from concourse.masks import make_identity  # noqa: E402

F32 = mybir.dt.float32
F32R = mybir.dt.float32r
F16 = mybir.dt.float16
U16 = mybir.dt.uint16
I16 = mybir.dt.int16
AF = mybir.ActivationFunctionType
OP = mybir.AluOpType

NCORES = 8
B = 4
N = 4096          # candidate points per batch
KNN = 32          # neighbors per knn
QPC = 2048        # query points per core
NT = 16           # query tiles of 128 per core
C1, C2, C3 = 64, 64, 128
NTOT = float(B * N * 2 * KNN)   # BN stat count (global)
BN_EPS = 1e-3
NEG = -1.0e30


def _pk(cc):
    """packed [128, 4096] slice coords for chunk cc."""
    return 64 * (cc % 2), 512 * (cc // 2)


def _r(ap):
    return ap.bitcast(F32R)


def _build_program(single=False):
    nc = bacc.Bacc(
        "TRN2", target_bir_lowering=False, debug=False,
        num_devices=1 if single else NCORES,
    )
    nc._single_core_nocoll = single

    ap = {}
    def din(name, shape, dt=F32):
        ap[name] = nc.dram_tensor(name, shape, dt, kind="ExternalInput").ap()
    din("qf", [4, QPC])
    din("gt", [128, N])
    din("qr", [4, QPC])
    din("qsq", [128, NT])
    din("w1t", [4, C1], F16)
    din("w2t", [128, C2], F16)     # duplicated at partition 64
    din("w3t", [128, C3], F16)     # duplicated at partition 64
    din("gt2", [128, N])
    din("gb1", [C1, 2])
    din("gb2", [C2, 2])
    din("gb3", [C3, 2])
    din("selw", [8, 128])

    ap["out"] = nc.dram_tensor("out", [3, QPC], F32, kind="ExternalOutput").ap()

    ap["y1d"] = nc.dram_tensor("y1d", [NT, 128, 4096], F16).ap()
    ap["y2d"] = nc.dram_tensor("y2d", [NT, 128, 4096], F16).ap()
    ap["y3d"] = nc.dram_tensor("y3d", [NT, C3, 8192], F16).ap()
    ap["g1d"] = nc.dram_tensor("g1d", [NT, 128, 512], F32).ap()
    ap["g2d"] = nc.dram_tensor("g2d", [NT, 128, 512], F32).ap()
    for i, c in ((0, C1), (1, C2), (2, C3)):
        ap[f"arin{i}"] = nc.dram_tensor(f"arin{i}", [c * 2], F32).ap()
        ap[f"arout{i}"] = nc.dram_tensor(f"arout{i}", [c * 2], F32).ap()

    with tile.TileContext(nc) as tc:
        _kernel_body(tc, ap)
    nc.compile()
    return nc


def _kernel_body(tc, d):
    nc = tc.nc
    from contextlib import ExitStack

    ctx = ExitStack()
    with ctx:
        # constants alive through the whole kernel
        cpool = ctx.enter_context(tc.tile_pool(name="consts", bufs=1))
        qf = cpool.tile([4, QPC], F32)
        qr = cpool.tile([4, QPC], F32)
        qsq = cpool.tile([128, NT], F32)
        w1 = cpool.tile([4, C1], F16)
        w2 = cpool.tile([128, C2], F16)
        w3 = cpool.tile([128, C3], F16)
        gb1 = cpool.tile([C1, 2], F32)
        gb2 = cpool.tile([C2, 2], F32)
        gb3 = cpool.tile([C3, 2], F32)
        selw = cpool.tile([8, 128], F32)
        ident = cpool.tile([128, 128], F32)
        make_identity(nc, ident[:])
        for nm, sb in [("qf", qf), ("qr", qr), ("qsq", qsq), ("w1t", w1),
                       ("w2t", w2), ("w3t", w3), ("gb1", gb1), ("gb2", gb2),
                       ("gb3", gb3), ("selw", selw)]:
            nc.sync.dma_start(out=sb[:], in_=d[nm][:])

        spool = ctx.enter_context(tc.tile_pool(name="stats", bufs=1))
        sm1 = spool.tile([C1, NT * 16], F32)
        sq1 = spool.tile([C1, NT * 16], F32)
        sm2 = spool.tile([C2, NT * 16], F32)
        sq2 = spool.tile([C2, NT * 16], F32)
        sm3 = spool.tile([C3, NT * 16], F32)
        sq3 = spool.tile([C3, NT * 16], F32)
        ab1 = spool.tile([128, 2], F32)   # col0 = scale a, col1 = bias b (dup at 64)
        ab2 = spool.tile([128, 2], F32)
        ab3 = spool.tile([C3, 2], F32)
        # qball[16g+c, t*16+p] = q coord c of query (t, g, p)
        qball = spool.tile([128, NT * 16], F32)

        # ---------------- Phase 1: knn + gather + feat + conv1 ----------------
        with tc.tile_pool(name="p1knn", bufs=1) as kpool, \
             tc.tile_pool(name="p1m", bufs=3) as mpool, \
             tc.tile_pool(name="p1psum", bufs=2, space="PSUM") as pp, \
             tc.tile_pool(name="p1tp", bufs=2, space="PSUM") as tpp, \
             tc.tile_pool(name="p1cpsum", bufs=3, space="PSUM") as cp, \
             tc.tile_pool(name="p1feat", bufs=1) as fpool, \
             tc.tile_pool(name="p1work", bufs=2) as wp, \
             tc.tile_pool(name="p1y", bufs=2) as yp:
            gt = kpool.tile([128, N], F32)
            gt2 = kpool.tile([128, N], F32)
            for nm, sb in [("gt", gt), ("gt2", gt2)]:
                nc.sync.dma_start(out=sb[:], in_=d[nm][:])
            for cc in range(3):
                nc.sync.dma_start(
                    out=qball[cc::16, :].rearrange("g (t p) -> g t p", t=NT),
                    in_=d["qr"][cc:cc + 1, :].rearrange(
                        "c (t g p) -> (c g) t p", t=NT, g=8))

            def emit_knn(t):
                vals = wp.tile([128, 64], F32, tag="vals")
                idxu = wp.tile([128, 64], U16, tag="idxu")
                idxi = wp.tile([128, 64], I16, tag="idxi")
                for kn, tab in ((0, gt), (1, gt2)):
                    msb = mpool.tile([128, N], F32, tag="msb")
                    # M = 2 q.c - |c|^2 (maximize == nearest)
                    for ch in range(8):
                        pm = pp.tile([128, 512], F32, tag="pm")
                        nc.tensor.matmul(
                            out=pm[:],
                            lhsT=qf[:, t * 128:(t + 1) * 128],
                            rhs=tab[0:4, ch * 512:(ch + 1) * 512],
                            start=True, stop=True,
                        )
                        nc.scalar.activation(
                            out=msb[:, ch * 512:(ch + 1) * 512], in_=pm[:],
                            func=AF.Identity)
                    # top-32 rounds
                    for r in range(4):
                        v8 = vals[:, kn * 32 + r * 8: kn * 32 + r * 8 + 8]
                        i8 = idxu[:, kn * 32 + r * 8: kn * 32 + r * 8 + 8]
                        nc.vector.max(out=v8, in_=msb[:])
                        nc.vector.max_index(out=i8, in_max=v8, in_values=msb[:])
                        if r < 3:
                            nc.vector.match_replace(
                                out=msb[:], in_to_replace=v8,
                                in_values=msb[:], imm_value=NEG)
                nc.vector.tensor_copy(out=idxi[:], in_=idxu[:])
                return vals, idxi

            def emit_post(t, vals, idxi):
                # gather neighbor coords; both tables carry xyz on band rows
                # 16g+{0..2} (gt = p1 for knn1, gt2 = p2 for knn2); spill raw
                # for the fusion phase
                g1 = wp.tile([128, 512], F32, tag="g1")
                g2 = wp.tile([128, 512], F32, tag="g2")
                nc.gpsimd.ap_gather(
                    out_ap=g1[:], in_ap=gt[:], idxs_ap=idxi[:, 0:32],
                    channels=128, num_elems=N, d=1, num_idxs=512)
                nc.gpsimd.ap_gather(
                    out_ap=g2[:], in_ap=gt2[:], idxs_ap=idxi[:, 32:64],
                    channels=128, num_elems=N, d=1, num_idxs=512)
                nc.sync.dma_start(out=d["g1d"][t], in_=g1[:])
                nc.sync.dma_start(out=d["g2d"][t], in_=g2[:])

                # resi = nn - q, in band layout (out of place to avoid WAR
                # with the raw spill)
                qb = qball[:, t * 16:(t + 1) * 16]
                g1r = wp.tile([128, 512], F16, tag="g1r")
                g2r = wp.tile([128, 512], F16, tag="g2r")
                for gsrc, gdst in ((g1, g1r), (g2, g2r)):
                    nc.vector.tensor_tensor(
                        out=gdst[:].rearrange("c (s p) -> c s p", s=32),
                        in0=gsrc[:].rearrange("c (s p) -> c s p", s=32),
                        in1=qb.unsqueeze(1).to_broadcast([128, 32, 16]),
                        op=OP.subtract)

                # conv1 rhs must start at partition 0: strided-partition DMAs
                # into a flat [4, 8192] tile (3 per table)
                feat = fpool.tile([4, 8192], F16, tag="feat")
                for kn, gsrc in ((0, g1r), (1, g2r)):
                    for cc in range(3):
                        nc.sync.dma_start(
                            out=feat[cc:cc + 1, kn * 4096:(kn + 1) * 4096]
                                .rearrange("c (g sp) -> c g sp", g=8),
                            in_=gsrc[cc::16, :])

                # dist = sqrt(relu(|q|^2 - val)) into feat row 3
                d2 = wp.tile([128, 64], F32, tag="d2")
                nc.scalar.activation(
                    out=d2[:], in_=vals[:], func=AF.Relu,
                    scale=-1.0, bias=qsq[:, t:t + 1])
                nc.scalar.activation(out=d2[:], in_=d2[:], func=AF.Sqrt)
                # shuffle dist to pixel layout: PE-transpose to [nbr, query],
                # then one DMA per knn half
                dtp = tpp.tile([64, 128], F32, tag="dtp")
                nc.tensor.transpose(out=dtp[:], in_=d2[:], identity=ident[:])
                d2t = wp.tile([64, 128], F16, tag="d2t")
                nc.scalar.activation(out=d2t[:], in_=dtp[:], func=AF.Identity)
                for kn in (0, 1):
                    for g in range(8):
                        c = kn * 8 + g
                        nc.sync.dma_start(
                            out=feat[3:4, c * 512:(c + 1) * 512]
                                .rearrange("c (s p) -> c s p", s=32),
                            in_=d2t[kn * 32:(kn + 1) * 32,
                                    16 * g:16 * g + 16])

                # conv1: 16 chunks -> y1 packed [128, 4096]
                y1 = yp.tile([128, 4096], F16, tag="y1")
                for c in range(16):
                    bp_, fo = _pk(c)
                    pc = cp.tile([C1, 512], F32, tag="pc1")
                    nc.tensor.matmul(
                        out=pc[:],
                        lhsT=w1[:],
                        rhs=feat[:, c * 512:(c + 1) * 512],
                        start=True, stop=True)
                    nc.scalar.activation(
                        out=y1[bp_:bp_ + 64, fo:fo + 512], in_=pc[:],
                        func=AF.Identity,
                        accum_out=sm1[:, t * 16 + c: t * 16 + c + 1])
                    sqs = wp.tile([C1, 512], F32, tag="sqs")
                    nc.scalar.activation(
                        out=sqs[:], in_=pc[:], func=AF.Square,
                        accum_out=sq1[:, t * 16 + c: t * 16 + c + 1])
                nc.sync.dma_start(out=d["y1d"][t], in_=y1[:])

            # software pipeline: tile t's dist matmuls + top-k are emitted
            # before tile t-1's gather/conv work so the DVE round stream and
            # the PE dist stream never wait on each other's tile.
            prev = None
            for t in range(NT):
                cur = emit_knn(t)
                if prev is not None:
                    emit_post(t - 1, *prev)
                prev = cur
            emit_post(NT - 1, *prev)

        _bn_allreduce(tc, 0, sm1, sq1, gb1, ab1, d["arin0"], d["arout0"], True)

        # ---------------- Phase 2: apply BN1+relu, conv2 ----------------
        with tc.tile_pool(name="p2y", bufs=2) as yp, \
             tc.tile_pool(name="p2psum", bufs=4, space="PSUM") as cp, \
             tc.tile_pool(name="p2work", bufs=2) as wp:
            for t in range(NT):
                y1 = yp.tile([128, 4096], F16, tag="y1l")
                nc.sync.dma_start(out=y1[:], in_=d["y1d"][t])
                nc.scalar.activation(
                    out=y1[:], in_=y1[:], func=AF.Relu,
                    scale=ab1[:, 0:1], bias=ab1[:, 1:2])
                y2 = yp.tile([128, 4096], F16, tag="y2")
                for c in range(16):
                    bp_, fo = _pk(c)
                    pc = cp.tile([C2, 512], F32, tag="pc2")
                    nc.tensor.matmul(
                        out=pc[:], lhsT=w2[bp_:bp_ + 64, :],
                        rhs=y1[bp_:bp_ + 64, fo:fo + 512],
                        start=True, stop=True)
                    nc.scalar.activation(
                        out=y2[bp_:bp_ + 64, fo:fo + 512], in_=pc[:],
                        func=AF.Identity,
                        accum_out=sm2[:, t * 16 + c: t * 16 + c + 1])
                    sqs = wp.tile([C2, 512], F32, tag="sqs2")
                    nc.scalar.activation(
                        out=sqs[:], in_=pc[:], func=AF.Square,
                        accum_out=sq2[:, t * 16 + c: t * 16 + c + 1])
                nc.sync.dma_start(out=d["y2d"][t], in_=y2[:])

        _bn_allreduce(tc, 1, sm2, sq2, gb2, ab2, d["arin1"], d["arout1"], True)

        # ---------------- Phase 3: apply BN2+relu, conv3 ----------------
        with tc.tile_pool(name="p3y", bufs=2) as yp, \
             tc.tile_pool(name="p3psum", bufs=4, space="PSUM") as cp, \
             tc.tile_pool(name="p3work", bufs=2) as wp:
            for t in range(NT):
                y2 = yp.tile([128, 4096], F16, tag="y2l")
                nc.sync.dma_start(out=y2[:], in_=d["y2d"][t])
                nc.scalar.activation(
                    out=y2[:], in_=y2[:], func=AF.Relu,
                    scale=ab2[:, 0:1], bias=ab2[:, 1:2])
                y3 = yp.tile([C3, 8192], F16, tag="y3")
                for c in range(16):
                    bp_, fo = _pk(c)
                    pc = cp.tile([C3, 512], F32, tag="pc3")
                    nc.tensor.matmul(
                        out=pc[:], lhsT=w3[bp_:bp_ + 64, :],
                        rhs=y2[bp_:bp_ + 64, fo:fo + 512],
                        start=True, stop=True)
                    nc.scalar.activation(
                        out=y3[:, c * 512:(c + 1) * 512], in_=pc[:],
                        func=AF.Identity,
                        accum_out=sm3[:, t * 16 + c: t * 16 + c + 1])
                    sqs = wp.tile([C3, 512], F32, tag="sqs3")
                    nc.scalar.activation(
                        out=sqs[:], in_=pc[:], func=AF.Square,
                        accum_out=sq3[:, t * 16 + c: t * 16 + c + 1])
                nc.sync.dma_start(out=d["y3d"][t], in_=y3[:])

        _bn_allreduce(tc, 2, sm3, sq3, gb3, ab3, d["arin2"], d["arout2"], False)

        # ------------- Phase 4: scores, softmax, fusion, output -------------
        with tc.tile_pool(name="p4y", bufs=2) as yp, \
             tc.tile_pool(name="p4sc", bufs=1) as scp, \
             tc.tile_pool(name="p4work", bufs=2) as wp, \
             tc.tile_pool(name="p4psum", bufs=2, space="PSUM") as pp4, \
             tc.tile_pool(name="p4out", bufs=1) as op_:
            outsb = op_.tile([4, QPC], F32)
            pscore = scp.tile([128, 8192], F32)
            for t in range(NT):
                y3 = yp.tile([C3, 8192], F16, tag="y3l")
                nc.sync.dma_start(out=y3[:], in_=d["y3d"][t])
                y3r = yp.tile([C3, 8192], F32, tag="y3r")
                nc.scalar.activation(
                    out=y3r[:], in_=y3[:], func=AF.Relu,
                    scale=ab3[:, 0:1], bias=ab3[:, 1:2])
                # channel-max scores via gpsimd partition reduce; batch the
                # row-0 extraction into one DMA per knn half
                for c in range(16):
                    nc.gpsimd.partition_all_reduce(
                        out_ap=pscore[:, c * 512:(c + 1) * 512],
                        in_ap=y3r[:, c * 512:(c + 1) * 512],
                        channels=128, reduce_op=bass_isa.ReduceOp.max)
                scA = wp.tile([8, 512], F32, tag="scA")
                scB = wp.tile([8, 512], F32, tag="scB")
                for kn, sct in ((0, scA), (1, scB)):
                    nc.sync.dma_start(
                        out=sct[:],
                        in_=pscore[0:1, kn * 4096:(kn + 1) * 4096]
                            .rearrange("c (g sp) -> c g sp", g=8))
                # softmax over the 64 neighbors of each query
                qmA = wp.tile([8, 16], F32, tag="qmA")
                qmB = wp.tile([8, 16], F32, tag="qmB")
                for sct, qm in ((scA, qmA), (scB, qmB)):
                    nc.vector.tensor_reduce(
                        out=qm[:],
                        in_=sct[:].rearrange("c (s p) -> c p s", s=32),
                        axis=mybir.AxisListType.X, op=OP.max)
                nc.vector.tensor_tensor(
                    out=qmA[:], in0=qmA[:], in1=qmB[:], op=OP.max)
                exA = wp.tile([8, 512], F32, tag="exA")
                exB = wp.tile([8, 512], F32, tag="exB")
                for sct, ext in ((scA, exA), (scB, exB)):
                    nc.vector.tensor_tensor(
                        out=ext[:].rearrange("c (s p) -> c s p", s=32),
                        in0=sct[:].rearrange("c (s p) -> c s p", s=32),
                        in1=qmA[:].unsqueeze(1).to_broadcast([8, 32, 16]),
                        op=OP.subtract)
                    nc.scalar.activation(out=ext[:], in_=ext[:], func=AF.Exp)
                esA = wp.tile([8, 16], F32, tag="esA")
                esB = wp.tile([8, 16], F32, tag="esB")
                for ext, est in ((exA, esA), (exB, esB)):
                    nc.vector.tensor_reduce(
                        out=est[:],
                        in_=ext[:].rearrange("c (s p) -> c p s", s=32),
                        axis=mybir.AxisListType.X, op=OP.add)
                nc.vector.tensor_tensor(
                    out=esA[:], in0=esA[:], in1=esB[:], op=OP.add)
                nc.vector.reciprocal(out=esA[:], in_=esA[:])
                for ext in (exA, exB):
                    nc.vector.tensor_tensor(
                        out=ext[:].rearrange("c (s p) -> c s p", s=32),
                        in0=ext[:].rearrange("c (s p) -> c s p", s=32),
                        in1=esA[:].unsqueeze(1).to_broadcast([8, 32, 16]),
                        op=OP.mult)
                # fusion: replicate weight rows onto band partitions via a
                # selector matmul, multiply with raw coords, segment-reduce
                g1 = wp.tile([128, 512], F32, tag="g1l")
                g2 = wp.tile([128, 512], F32, tag="g2l")
                nc.sync.dma_start(out=g1[:], in_=d["g1d"][t])
                nc.sync.dma_start(out=g2[:], in_=d["g2d"][t])
                wr1 = wp.tile([128, 512], F32, tag="wr1")
                wr2 = wp.tile([128, 512], F32, tag="wr2")
                for ext, wr in ((exA, wr1), (exB, wr2)):
                    pw = pp4.tile([128, 512], F32, tag="pw")
                    nc.tensor.matmul(
                        out=pw[:], lhsT=selw[:],
                        rhs=ext[:], start=True, stop=True)
                    nc.scalar.activation(out=wr[:], in_=pw[:], func=AF.Identity)
                pr = wp.tile([128, 512], F32, tag="pr")
                nc.vector.tensor_tensor(out=pr[:], in0=g1[:], in1=wr1[:],
                                        op=OP.mult)
                nc.vector.tensor_tensor(out=wr2[:], in0=g2[:], in1=wr2[:],
                                        op=OP.mult)
                nc.vector.tensor_tensor(out=pr[:], in0=pr[:], in1=wr2[:],
                                        op=OP.add)
                fp = wp.tile([128, 16], F32, tag="fp")
                nc.vector.tensor_reduce(
                    out=fp[:], in_=pr[:].rearrange("c (s p) -> c p s", s=32),
                    axis=mybir.AxisListType.X, op=OP.add)
                for cc in range(3):
                    nc.sync.dma_start(
                        out=outsb[cc:cc + 1, t * 128:(t + 1) * 128]
                            .rearrange("c (g p) -> c g p", g=8),
                        in_=fp[cc::16, :])
            nc.sync.dma_start(out=d["out"][:], in_=outsb[0:3, :])


def _bn_allreduce(tc, li, sm, sq, gbe, ab, arin, arout, dup):
    """Reduce per-chunk stat slots, AllReduce across 8 cores, compute
    per-channel scale a = g*rsqrt(var+eps) and bias b = be - a*mean."""
    nc = tc.nc
    C = sm.shape[0]
    with tc.tile_pool(name=f"bn{li}", bufs=1) as bp:
        st = bp.tile([C, 2], F32)
        nc.vector.tensor_reduce(out=st[:, 0:1], in_=sm[:],
                                axis=mybir.AxisListType.X, op=OP.add)
        nc.vector.tensor_reduce(out=st[:, 1:2], in_=sq[:],
                                axis=mybir.AxisListType.X, op=OP.add)
        nc.sync.dma_start(out=arin[:], in_=st[:])
        if getattr(nc, "_single_core_nocoll", False):
            nc.sync.dma_start(out=arout[:], in_=arin[:])
        else:
            nc.gpsimd.collective_compute(
                "AllReduce", OP.add, replica_groups=[list(range(NCORES))],
                ins=[arin.opt()], outs=[arout.opt()])
        ar = bp.tile([C, 2], F32)
        nc.sync.dma_start(out=ar[:], in_=arout[:])
        mean = bp.tile([C, 1], F32)
        var = bp.tile([C, 1], F32)
        nc.vector.tensor_scalar_mul(mean[:], ar[:, 0:1], 1.0 / NTOT)
        nc.vector.tensor_scalar_mul(var[:], ar[:, 1:2], 1.0 / NTOT)
        m2 = bp.tile([C, 1], F32)
        nc.vector.tensor_tensor(out=m2[:], in0=mean[:], in1=mean[:], op=OP.mult)
        nc.vector.tensor_tensor(out=var[:], in0=var[:], in1=m2[:], op=OP.subtract)
        nc.vector.tensor_scalar_add(var[:], var[:], BN_EPS)
        nc.scalar.activation(out=var[:], in_=var[:], func=AF.Sqrt)
        nc.vector.reciprocal(out=var[:], in_=var[:])  # rsqrt(var+eps)
        nc.vector.tensor_tensor(out=ab[0:C, 0:1], in0=var[:], in1=gbe[:, 0:1],
                                op=OP.mult)            # a
        nc.vector.tensor_tensor(out=m2[:], in0=ab[0:C, 0:1], in1=mean[:],
                                op=OP.mult)
        nc.vector.tensor_tensor(out=ab[0:C, 1:2], in0=gbe[:, 1:2], in1=m2[:],
                                op=OP.subtract)        # b = be - a*mean
        if dup:
            nc.vector.tensor_copy(out=ab[C:2 * C, :], in_=ab[0:C, :])


_PROGRAM = None
LAST_RESULT = None


def _get_program():
    global _PROGRAM
    if _PROGRAM is None:
        _PROGRAM = _build_program()
    return _PROGRAM


def _prep_core_inputs(points1, points2, W1, W2, W3, gs, bes, b, h):
    p1 = points1[b]          # [3, N]
    p2 = points2[b]
    q = p1[:, h * QPC:(h + 1) * QPC]            # [3, QPC]
    qf = np.concatenate([2.0 * q, np.ones((1, QPC), np.float32)], axis=0)

    # rows 16g+{0..2}: coords (gather bands); row 3: -|c|^2 (dist matmul rhs
    # reads rows 0:4, and band-0 row 3 is never gathered)
    gtab = np.zeros((128, N), np.float32)
    gtab2 = np.zeros((128, N), np.float32)
    for g in range(8):
        gtab[16 * g + 0:16 * g + 3] = p1
        gtab2[16 * g + 0:16 * g + 3] = p2
    gtab[3] = -(p1 * p1).sum(axis=0)
    gtab2[3] = -(p2 * p2).sum(axis=0)
    qraw = np.zeros((4, QPC), np.float32)
    qraw[0:3] = q
    qsqv = (q * q).sum(axis=0).reshape(NT, 128).T.astype(np.float32)  # [128, NT]

    def dup128(w):      # [64, C] -> [128, C] duplicated
        return np.concatenate([w, w], axis=0).astype(np.float16)

    selw = np.zeros((8, 128), np.float32)
    for g in range(8):
        for c3 in range(3):
            selw[g, 16 * g + c3] = 1.0

    return {
        "selw": selw,
        "qf": qf.astype(np.float32),
        "gt": gtab, "gt2": gtab2,
        "qr": qraw, "qsq": np.ascontiguousarray(qsqv),
        "w1t": np.ascontiguousarray(W1.T).astype(np.float16),
        "w2t": dup128(np.ascontiguousarray(W2.T)),
        "w3t": dup128(np.ascontiguousarray(W3.T)),
        "gb1": np.stack([gs[0], bes[0]], axis=1).astype(np.float32),
        "gb2": np.stack([gs[1], bes[1]], axis=1).astype(np.float32),
        "gb3": np.stack([gs[2], bes[2]], axis=1).astype(np.float32),
    }


def kernel(points1, points2, k, t, W1, b1, g1, be1, W2, b2, g2, be2,
           W3, b3, g3, be3):
    # b1/b2/b3 cancel inside train-mode BatchNorm; t is unused by the net.
    assert int(np.asarray(k)) == KNN
    points1 = np.asarray(points1, np.float32)
    points2 = np.asarray(points2, np.float32)
    gs = [np.asarray(g1, np.float32), np.asarray(g2, np.float32),
          np.asarray(g3, np.float32)]
    bes = [np.asarray(be1, np.float32), np.asarray(be2, np.float32),
           np.asarray(be3, np.float32)]
    Ws = [np.asarray(W1, np.float32), np.asarray(W2, np.float32),
          np.asarray(W3, np.float32)]

    in_maps = []
    for c in range(NCORES):
        b, h = divmod(c, 2)
        in_maps.append(_prep_core_inputs(points1, points2, *Ws, gs, bes, b, h))

    nc = _get_program()
    bkr = run_bass_kernel_spmd(nc, in_maps, list(range(NCORES)))
    global LAST_RESULT
    LAST_RESULT = bkr
    res = bkr.results

    out = np.zeros((B, 3, N), np.float32)
    for c in range(NCORES):
        b, h = divmod(c, 2)
        out[b, :, h * QPC:(h + 1) * QPC] = res[c]["out"]
    return out


# revision 15
# speedup vs baseline: 1.4304x; 1.1527x over previous
"""PointsFusion Trainium2 kernel.

Pipeline per batch b (B=4, N=4096, k=32):
  knn1 = 32-NN of p1 in p1, knn2 = 32-NN of p1 in p2 (exact, via DVE 8-max rounds)
  gather neighbor coords, features (resi, dist) -> conv(4->64)->BN->relu
  -> conv(64->64)->BN->relu -> conv(64->128)->BN->relu -> channel-max scores
  -> softmax over 64 neighbors -> weighted sum of neighbor coords.

Sharding: 8 cores = (batch b, half h of the 4096 query points). BatchNorm uses
global batch stats -> 3 tiny AllReduces of per-channel sum/sumsq.

Layouts (per 128-query tile):
  pixel space: 16 chunks of 512; chunk c = kn*8+g, pixel j = c*512 + s*16 + p
  (g = query group, p = query-in-group, s = neighbor slot, kn = which knn).
  64-channel activations are packed [128, 4096]: chunk c lives at partitions
  64*(c%2)..+64, free 512*(c//2)..+512 (keeps matmul rhs bases in {0, 64}).

Self-contained: hardcodes shapes; no sibling imports.
"""

import sys

import numpy as np

for _p in ("/opt/trn_rl_repo", "/opt/pypackages"):
    if _p not in sys.path:
        sys.path.append(_p)

import concourse.bass as bass  # noqa: E402  (imported for side effects/typing)
import concourse.mybir as mybir  # noqa: E402
import concourse.tile as tile  # noqa: E402
from concourse import bacc, bass_isa  # noqa: E402
from concourse.bass_utils import run_bass_kernel_spmd  # noqa: E402
from concourse.masks import make_identity  # noqa: E402

F32 = mybir.dt.float32
F32R = mybir.dt.float32r
F16 = mybir.dt.float16
U16 = mybir.dt.uint16
I16 = mybir.dt.int16
AF = mybir.ActivationFunctionType
OP = mybir.AluOpType

NCORES = 8
B = 4
N = 4096          # candidate points per batch
KNN = 32          # neighbors per knn
QPC = 2048        # query points per core
NT = 16           # query tiles of 128 per core
C1, C2, C3 = 64, 64, 128
NTOT = float(B * N * 2 * KNN)   # BN stat count (global)
BN_EPS = 1e-3
NEG = -1.0e30


def _pk(cc):
    """packed [128, 4096] slice coords for chunk cc."""
    return 64 * (cc % 2), 512 * (cc // 2)


def _r(ap):
    return ap.bitcast(F32R)


def _build_program(single=False):
    nc = bacc.Bacc(
        "TRN2", target_bir_lowering=False, debug=False,
        num_devices=1 if single else NCORES,
    )
    nc._single_core_nocoll = single

    ap = {}
    def din(name, shape, dt=F32):
        ap[name] = nc.dram_tensor(name, shape, dt, kind="ExternalInput").ap()
    din("qf", [4, QPC])
    din("gt", [128, N])
    din("qr", [4, QPC])
    din("qsq", [128, NT])
    din("w1t", [4, C1], F16)
    din("w2t", [128, C2], F16)     # duplicated at partition 64
    din("w3t", [128, C3], F16)     # duplicated at partition 64
    din("gt2", [128, N])
    din("gb1", [C1, 2])
    din("gb2", [C2, 2])
    din("gb3", [C3, 2])
    din("selw", [8, 128])
    din("w2f", [C1, C2])
    din("w3f", [C2, C3])

    ap["out"] = nc.dram_tensor("out", [3, QPC], F32, kind="ExternalOutput").ap()

    ap["y1d"] = nc.dram_tensor("y1d", [NT, 128, 4096], F16).ap()
    ap["y2d"] = nc.dram_tensor("y2d", [NT, 128, 4096], F16).ap()
    ap["y3d"] = nc.dram_tensor("y3d", [NT, C3, 8192], F16).ap()
    ap["g1d"] = nc.dram_tensor("g1d", [NT, 128, 512], F32).ap()
    ap["g2d"] = nc.dram_tensor("g2d", [NT, 128, 512], F32).ap()
    for i, c in ((0, C1), (1, C2), (2, C3)):
        ap[f"arin{i}"] = nc.dram_tensor(f"arin{i}", [c * 2], F32).ap()
        ap[f"arout{i}"] = nc.dram_tensor(f"arout{i}", [c * 2], F32).ap()

    with tile.TileContext(nc) as tc:
        _kernel_body(tc, ap)
    nc.compile()
    return nc


def _kernel_body(tc, d):
    nc = tc.nc
    from contextlib import ExitStack

    ctx = ExitStack()
    with ctx:
        # constants alive through the whole kernel
        cpool = ctx.enter_context(tc.tile_pool(name="consts", bufs=1))
        qf = cpool.tile([4, QPC], F32)
        qr = cpool.tile([4, QPC], F32)
        qsq = cpool.tile([128, NT], F32)
        w1 = cpool.tile([4, C1], F16)
        w2 = cpool.tile([128, C2], F16)
        w3 = cpool.tile([128, C3], F16)
        gb1 = cpool.tile([C1, 2], F32)
        gb2 = cpool.tile([C2, 2], F32)
        gb3 = cpool.tile([C3, 2], F32)
        selw = cpool.tile([8, 128], F32)
        w2f = cpool.tile([C1, C2], F32)
        w3f = cpool.tile([C2, C3], F32)
        ident = cpool.tile([128, 128], F32)
        make_identity(nc, ident[:])
        for nm, sb in [("qf", qf), ("qr", qr), ("qsq", qsq), ("w1t", w1),
                       ("w2t", w2), ("w3t", w3), ("gb1", gb1), ("gb2", gb2),
                       ("gb3", gb3), ("selw", selw), ("w2f", w2f),
                       ("w3f", w3f)]:
            nc.sync.dma_start(out=sb[:], in_=d[nm][:])

        spool = ctx.enter_context(tc.tile_pool(name="stats", bufs=1))
        sm1 = spool.tile([C1, NT * 16], F32)
        sq1 = spool.tile([C1, NT * 16], F32)
        sxa2 = spool.tile([128, NT], F32)
        sqp2 = spool.tile([128, NT * 8], F32)
        sxa3 = spool.tile([128, NT], F32)
        sqp3 = spool.tile([C3, NT * 16], F32)
        ab1 = spool.tile([128, 2], F32)   # col0 = scale a, col1 = bias b (dup at 64)
        ab2 = spool.tile([128, 2], F32)
        ab3 = spool.tile([C3, 2], F32)
        # qball[16g+c, t*16+p] = q coord c of query (t, g, p)
        qball = spool.tile([128, NT * 16], F32)

        # ---------------- Phase 1: knn + gather + feat + conv1 ----------------
        with tc.tile_pool(name="p1knn", bufs=1) as kpool, \
             tc.tile_pool(name="p1m", bufs=3) as mpool, \
             tc.tile_pool(name="p1psum", bufs=2, space="PSUM") as pp, \
             tc.tile_pool(name="p1tp", bufs=2, space="PSUM") as tpp, \
             tc.tile_pool(name="p1cpsum", bufs=3, space="PSUM") as cp, \
             tc.tile_pool(name="p1feat", bufs=1) as fpool, \
             tc.tile_pool(name="p1work", bufs=2) as wp, \
             tc.tile_pool(name="p1y", bufs=2) as yp:
            gt = kpool.tile([128, N], F32)
            gt2 = kpool.tile([128, N], F32)
            for nm, sb in [("gt", gt), ("gt2", gt2)]:
                nc.sync.dma_start(out=sb[:], in_=d[nm][:])
            for cc in range(3):
                nc.sync.dma_start(
                    out=qball[cc::16, :].rearrange("g (t p) -> g t p", t=NT),
                    in_=d["qr"][cc:cc + 1, :].rearrange(
                        "c (t g p) -> (c g) t p", t=NT, g=8))

            def emit_knn(t):
                vals = wp.tile([128, 64], F32, tag="vals")
                idxu = wp.tile([128, 64], U16, tag="idxu")
                idxi = wp.tile([128, 64], I16, tag="idxi")
                for kn, tab in ((0, gt), (1, gt2)):
                    msb = mpool.tile([128, N], F32, tag="msb")
                    # M = 2 q.c - |c|^2 (maximize == nearest)
                    for ch in range(8):
                        pm = pp.tile([128, 512], F32, tag="pm")
                        nc.tensor.matmul(
                            out=pm[:],
                            lhsT=qf[:, t * 128:(t + 1) * 128],
                            rhs=tab[0:4, ch * 512:(ch + 1) * 512],
                            start=True, stop=True,
                        )
                        nc.scalar.activation(
                            out=msb[:, ch * 512:(ch + 1) * 512], in_=pm[:],
                            func=AF.Identity)
                    # top-32 rounds
                    for r in range(4):
                        v8 = vals[:, kn * 32 + r * 8: kn * 32 + r * 8 + 8]
                        i8 = idxu[:, kn * 32 + r * 8: kn * 32 + r * 8 + 8]
                        nc.vector.max(out=v8, in_=msb[:])
                        nc.vector.max_index(out=i8, in_max=v8, in_values=msb[:])
                        if r < 3:
                            nc.vector.match_replace(
                                out=msb[:], in_to_replace=v8,
                                in_values=msb[:], imm_value=NEG)
                nc.vector.tensor_copy(out=idxi[:], in_=idxu[:])
                return vals, idxi

            def emit_post(t, vals, idxi):
                # gather neighbor coords; both tables carry xyz on band rows
                # 16g+{0..2} (gt = p1 for knn1, gt2 = p2 for knn2); spill raw
                # for the fusion phase
                g1 = wp.tile([128, 512], F32, tag="g1")
                g2 = wp.tile([128, 512], F32, tag="g2")
                nc.gpsimd.ap_gather(
                    out_ap=g1[:], in_ap=gt[:], idxs_ap=idxi[:, 0:32],
                    channels=128, num_elems=N, d=1, num_idxs=512)
                nc.gpsimd.ap_gather(
                    out_ap=g2[:], in_ap=gt2[:], idxs_ap=idxi[:, 32:64],
                    channels=128, num_elems=N, d=1, num_idxs=512)
                nc.sync.dma_start(out=d["g1d"][t], in_=g1[:])
                nc.sync.dma_start(out=d["g2d"][t], in_=g2[:])

                # resi = nn - q, in band layout (out of place to avoid WAR
                # with the raw spill)
                qb = qball[:, t * 16:(t + 1) * 16]
                g1r = wp.tile([128, 512], F16, tag="g1r")
                g2r = wp.tile([128, 512], F16, tag="g2r")
                for gsrc, gdst in ((g1, g1r), (g2, g2r)):
                    nc.vector.tensor_tensor(
                        out=gdst[:].rearrange("c (s p) -> c s p", s=32),
                        in0=gsrc[:].rearrange("c (s p) -> c s p", s=32),
                        in1=qb.unsqueeze(1).to_broadcast([128, 32, 16]),
                        op=OP.subtract)

                # conv1 rhs must start at partition 0: strided-partition DMAs
                # into a flat [4, 8192] tile (3 per table)
                feat = fpool.tile([4, 8192], F16, tag="feat")
                for kn, gsrc in ((0, g1r), (1, g2r)):
                    for cc in range(3):
                        nc.sync.dma_start(
                            out=feat[cc:cc + 1, kn * 4096:(kn + 1) * 4096]
                                .rearrange("c (g sp) -> c g sp", g=8),
                            in_=gsrc[cc::16, :])

                # dist = sqrt(relu(|q|^2 - val)) into feat row 3
                d2 = wp.tile([128, 64], F32, tag="d2")
                nc.scalar.activation(
                    out=d2[:], in_=vals[:], func=AF.Relu,
                    scale=-1.0, bias=qsq[:, t:t + 1])
                nc.scalar.activation(out=d2[:], in_=d2[:], func=AF.Sqrt)
                # shuffle dist to pixel layout: PE-transpose to [nbr, query],
                # then one DMA per knn half
                dtp = tpp.tile([64, 128], F32, tag="dtp")
                nc.tensor.transpose(out=dtp[:], in_=d2[:], identity=ident[:])
                d2t = wp.tile([64, 128], F16, tag="d2t")
                nc.scalar.activation(out=d2t[:], in_=dtp[:], func=AF.Identity)
                for kn in (0, 1):
                    for g in range(8):
                        c = kn * 8 + g
                        nc.sync.dma_start(
                            out=feat[3:4, c * 512:(c + 1) * 512]
                                .rearrange("c (s p) -> c s p", s=32),
                            in_=d2t[kn * 32:(kn + 1) * 32,
                                    16 * g:16 * g + 16])

                # conv1: 16 chunks -> y1 packed [128, 4096]
                y1 = yp.tile([128, 4096], F16, tag="y1")
                for c in range(16):
                    bp_, fo = _pk(c)
                    pc = cp.tile([C1, 512], F32, tag="pc1")
                    nc.tensor.matmul(
                        out=pc[:],
                        lhsT=w1[:],
                        rhs=feat[:, c * 512:(c + 1) * 512],
                        start=True, stop=True)
                    nc.scalar.activation(
                        out=y1[bp_:bp_ + 64, fo:fo + 512], in_=pc[:],
                        func=AF.Identity,
                        accum_out=sm1[:, t * 16 + c: t * 16 + c + 1])
                    sqs = wp.tile([C1, 512], F32, tag="sqs")
                    nc.scalar.activation(
                        out=sqs[:], in_=pc[:], func=AF.Square,
                        accum_out=sq1[:, t * 16 + c: t * 16 + c + 1])
                nc.sync.dma_start(out=d["y1d"][t], in_=y1[:])

            # software pipeline: tile t's dist matmuls + top-k are emitted
            # before tile t-1's gather/conv work so the DVE round stream and
            # the PE dist stream never wait on each other's tile.
            prev = None
            for t in range(NT):
                cur = emit_knn(t)
                if prev is not None:
                    emit_post(t - 1, *prev)
                prev = cur
            emit_post(NT - 1, *prev)

        _bn_allreduce(tc, 0, sm1, sq1, gb1, ab1, d["arin0"], d["arout0"], True)

        # ---------------- Phase 2: apply BN1+relu, conv2 ----------------
        with tc.tile_pool(name="p2y", bufs=2) as yp, \
             tc.tile_pool(name="p2psum", bufs=4, space="PSUM") as cp, \
             tc.tile_pool(name="p2work", bufs=2) as wp:
            for t in range(NT):
                y1 = yp.tile([128, 4096], F16, tag="y1l")
                nc.sync.dma_start(out=y1[:], in_=d["y1d"][t])
                nc.scalar.activation(
                    out=y1[:], in_=y1[:], func=AF.Relu,
                    scale=ab1[:, 0:1], bias=ab1[:, 1:2],
                    accum_out=sxa2[:, t:t + 1])
                y2 = yp.tile([128, 4096], F16, tag="y2")
                for c in range(16):
                    bp_, fo = _pk(c)
                    pc = cp.tile([C2, 512], F32, tag="pc2")
                    nc.tensor.matmul(
                        out=pc[:], lhsT=w2[bp_:bp_ + 64, :],
                        rhs=y1[bp_:bp_ + 64, fo:fo + 512],
                        start=True, stop=True)
                    nc.scalar.activation(
                        out=y2[bp_:bp_ + 64, fo:fo + 512], in_=pc[:],
                        func=AF.Identity)
                nc.sync.dma_start(out=d["y2d"][t], in_=y2[:])
                ysq = wp.tile([128, 4096], F16, tag="ysq2")
                nc.vector.tensor_tensor(out=ysq[:], in0=y2[:], in1=y2[:],
                                        op=OP.mult)
                nc.vector.tensor_reduce(
                    out=sqp2[:, t * 8:(t + 1) * 8],
                    in_=ysq[:].rearrange("c (h f) -> c h f", h=8),
                    axis=mybir.AxisListType.X, op=OP.add)

        _bn_finalize23(tc, 1, sxa2, sqp2, w2f, gb2, ab2, d["arin1"],
                       d["arout1"], dup=True)

        # ---------------- Phase 3: apply BN2+relu, conv3 ----------------
        with tc.tile_pool(name="p3y", bufs=2) as yp, \
             tc.tile_pool(name="p3psum", bufs=4, space="PSUM") as cp, \
             tc.tile_pool(name="p3work", bufs=2) as wp:
            for t in range(NT):
                y2 = yp.tile([128, 4096], F16, tag="y2l")
                nc.sync.dma_start(out=y2[:], in_=d["y2d"][t])
                nc.scalar.activation(
                    out=y2[:], in_=y2[:], func=AF.Relu,
                    scale=ab2[:, 0:1], bias=ab2[:, 1:2],
                    accum_out=sxa3[:, t:t + 1])
                y3 = yp.tile([C3, 8192], F16, tag="y3")
                for c in range(16):
                    bp_, fo = _pk(c)
                    pc = cp.tile([C3, 512], F32, tag="pc3")
                    nc.tensor.matmul(
                        out=pc[:], lhsT=w3[bp_:bp_ + 64, :],
                        rhs=y2[bp_:bp_ + 64, fo:fo + 512],
                        start=True, stop=True)
                    nc.scalar.activation(
                        out=y3[:, c * 512:(c + 1) * 512], in_=pc[:],
                        func=AF.Identity)
                nc.sync.dma_start(out=d["y3d"][t], in_=y3[:])
                ysq = wp.tile([C3, 8192], F16, tag="ysq3")
                nc.vector.tensor_tensor(out=ysq[:], in0=y3[:], in1=y3[:],
                                        op=OP.mult)
                nc.vector.tensor_reduce(
                    out=sqp3[:, t * 16:(t + 1) * 16],
                    in_=ysq[:].rearrange("c (h f) -> c h f", h=16),
                    axis=mybir.AxisListType.X, op=OP.add)

        _bn_finalize23(tc, 2, sxa3, sqp3, w3f, gb3, ab3, d["arin2"],
                       d["arout2"], dup=False)

        # ------------- Phase 4: scores, softmax, fusion, output -------------
        with tc.tile_pool(name="p4y", bufs=2) as yp, \
             tc.tile_pool(name="p4sc", bufs=1) as scp, \
             tc.tile_pool(name="p4work", bufs=2) as wp, \
             tc.tile_pool(name="p4psum", bufs=2, space="PSUM") as pp4, \
             tc.tile_pool(name="p4out", bufs=1) as op_:
            outsb = op_.tile([4, QPC], F32)
            pscore = scp.tile([128, 8192], F32)
            for t in range(NT):
                y3 = yp.tile([C3, 8192], F16, tag="y3l")
                nc.sync.dma_start(out=y3[:], in_=d["y3d"][t])
                y3r = yp.tile([C3, 8192], F32, tag="y3r")
                nc.scalar.activation(
                    out=y3r[:], in_=y3[:], func=AF.Relu,
                    scale=ab3[:, 0:1], bias=ab3[:, 1:2])
                # channel-max scores via gpsimd partition reduce; batch the
                # row-0 extraction into one DMA per knn half
                for c in range(16):
                    nc.gpsimd.partition_all_reduce(
                        out_ap=pscore[:, c * 512:(c + 1) * 512],
                        in_ap=y3r[:, c * 512:(c + 1) * 512],
                        channels=128, reduce_op=bass_isa.ReduceOp.max)
                scA = wp.tile([8, 512], F32, tag="scA")
                scB = wp.tile([8, 512], F32, tag="scB")
                for kn, sct in ((0, scA), (1, scB)):
                    nc.sync.dma_start(
                        out=sct[:],
                        in_=pscore[0:1, kn * 4096:(kn + 1) * 4096]
                            .rearrange("c (g sp) -> c g sp", g=8))
                # softmax over the 64 neighbors of each query
                qmA = wp.tile([8, 16], F32, tag="qmA")
                qmB = wp.tile([8, 16], F32, tag="qmB")
                for sct, qm in ((scA, qmA), (scB, qmB)):
                    nc.vector.tensor_reduce(
                        out=qm[:],
                        in_=sct[:].rearrange("c (s p) -> c p s", s=32),
                        axis=mybir.AxisListType.X, op=OP.max)
                nc.vector.tensor_tensor(
                    out=qmA[:], in0=qmA[:], in1=qmB[:], op=OP.max)
                exA = wp.tile([8, 512], F32, tag="exA")
                exB = wp.tile([8, 512], F32, tag="exB")
                for sct, ext in ((scA, exA), (scB, exB)):
                    nc.vector.tensor_tensor(
                        out=ext[:].rearrange("c (s p) -> c s p", s=32),
                        in0=sct[:].rearrange("c (s p) -> c s p", s=32),
                        in1=qmA[:].unsqueeze(1).to_broadcast([8, 32, 16]),
                        op=OP.subtract)
                    nc.scalar.activation(out=ext[:], in_=ext[:], func=AF.Exp)
                esA = wp.tile([8, 16], F32, tag="esA")
                esB = wp.tile([8, 16], F32, tag="esB")
                for ext, est in ((exA, esA), (exB, esB)):
                    nc.vector.tensor_reduce(
                        out=est[:],
                        in_=ext[:].rearrange("c (s p) -> c p s", s=32),
                        axis=mybir.AxisListType.X, op=OP.add)
                nc.vector.tensor_tensor(
                    out=esA[:], in0=esA[:], in1=esB[:], op=OP.add)
                nc.vector.reciprocal(out=esA[:], in_=esA[:])
                for ext in (exA, exB):
                    nc.vector.tensor_tensor(
                        out=ext[:].rearrange("c (s p) -> c s p", s=32),
                        in0=ext[:].rearrange("c (s p) -> c s p", s=32),
                        in1=esA[:].unsqueeze(1).to_broadcast([8, 32, 16]),
                        op=OP.mult)
                # fusion: replicate weight rows onto band partitions via a
                # selector matmul, multiply with raw coords, segment-reduce
                g1 = wp.tile([128, 512], F32, tag="g1l")
                g2 = wp.tile([128, 512], F32, tag="g2l")
                nc.sync.dma_start(out=g1[:], in_=d["g1d"][t])
                nc.sync.dma_start(out=g2[:], in_=d["g2d"][t])
                wr1 = wp.tile([128, 512], F32, tag="wr1")
                wr2 = wp.tile([128, 512], F32, tag="wr2")
                for ext, wr in ((exA, wr1), (exB, wr2)):
                    pw = pp4.tile([128, 512], F32, tag="pw")
                    nc.tensor.matmul(
                        out=pw[:], lhsT=selw[:],
                        rhs=ext[:], start=True, stop=True)
                    nc.scalar.activation(out=wr[:], in_=pw[:], func=AF.Identity)
                pr = wp.tile([128, 512], F32, tag="pr")
                nc.vector.tensor_tensor(out=pr[:], in0=g1[:], in1=wr1[:],
                                        op=OP.mult)
                nc.vector.tensor_tensor(out=wr2[:], in0=g2[:], in1=wr2[:],
                                        op=OP.mult)
                nc.vector.tensor_tensor(out=pr[:], in0=pr[:], in1=wr2[:],
                                        op=OP.add)
                fp = wp.tile([128, 16], F32, tag="fp")
                nc.vector.tensor_reduce(
                    out=fp[:], in_=pr[:].rearrange("c (s p) -> c p s", s=32),
                    axis=mybir.AxisListType.X, op=OP.add)
                for cc in range(3):
                    nc.sync.dma_start(
                        out=outsb[cc:cc + 1, t * 128:(t + 1) * 128]
                            .rearrange("c (g p) -> c g p", g=8),
                        in_=fp[cc::16, :])
            nc.sync.dma_start(out=d["out"][:], in_=outsb[0:3, :])


def _bn_finalize23(tc, li, sxa, sqp, wf, gbe, ab, arin, arout, dup):
    """BN stats for conv2/conv3: sum(y) = W @ sum(x) (sum(x) from the relu
    pass accums), sum(y^2) from the DVE per-tile partials. AllReduce packs
    [sum(x) | sum(y^2)] as one [C, 2] tile."""
    nc = tc.nc
    Cin = 64
    Cout = wf.shape[1]
    with tc.tile_pool(name=f"bnf{li}", bufs=1) as bp, \
         tc.tile_pool(name=f"bnfp{li}", bufs=1, space="PSUM") as pp:
        st = bp.tile([Cout, 2], F32)
        if Cout > Cin:
            nc.vector.tensor_scalar_mul(st[:, 0:1], st[:, 0:1], 0.0)
        red = bp.tile([128, 2], F32)
        hi = bp.tile([64, 2], F32)
        nc.vector.tensor_reduce(out=red[:, 0:1], in_=sxa[:],
                                axis=mybir.AxisListType.X, op=OP.add)
        nc.vector.tensor_reduce(out=red[0:sqp.shape[0], 1:2], in_=sqp[:],
                                axis=mybir.AxisListType.X, op=OP.add)
        # fold the packed halves (DVE needs equal partition bases -> bounce
        # the upper half through a base-0 tile)
        nc.vector.tensor_copy(out=hi[:], in_=red[64:128, :])
        nc.vector.tensor_tensor(out=st[0:Cin, 0:1], in0=red[0:64, 0:1],
                                in1=hi[:, 0:1], op=OP.add)
        if Cout == 64:   # packed couts: fold halves
            nc.vector.tensor_tensor(out=st[:, 1:2], in0=red[0:64, 1:2],
                                    in1=hi[:, 1:2], op=OP.add)
        else:
            nc.vector.tensor_copy(out=st[:, 1:2], in_=red[:, 1:2])
        nc.sync.dma_start(out=arin[:], in_=st[:])
        if getattr(nc, "_single_core_nocoll", False):
            nc.sync.dma_start(out=arout[:], in_=arin[:])
        else:
            nc.gpsimd.collective_compute(
                "AllReduce", OP.add, replica_groups=[list(range(NCORES))],
                ins=[arin.opt()], outs=[arout.opt()])
        ar = bp.tile([Cout, 2], F32)
        nc.sync.dma_start(out=ar[:], in_=arout[:])
        # sum(y) = W @ sum(x): lhsT = W^T [Cin, Cout]
        ps = pp.tile([Cout, 1], F32)
        nc.tensor.matmul(out=ps[:], lhsT=wf[:], rhs=ar[0:Cin, 0:1],
                         start=True, stop=True)
        mean = bp.tile([Cout, 1], F32)
        nc.scalar.activation(out=mean[:], in_=ps[:], func=AF.Copy,
                             scale=1.0 / NTOT)
        var = bp.tile([Cout, 1], F32)
        nc.vector.tensor_scalar_mul(var[:], ar[:, 1:2], 1.0 / NTOT)
        m2 = bp.tile([Cout, 1], F32)
        nc.vector.tensor_tensor(out=m2[:], in0=mean[:], in1=mean[:], op=OP.mult)
        nc.vector.tensor_tensor(out=var[:], in0=var[:], in1=m2[:],
                                op=OP.subtract)
        nc.vector.tensor_scalar_add(var[:], var[:], BN_EPS)
        nc.scalar.activation(out=var[:], in_=var[:], func=AF.Sqrt)
        nc.vector.reciprocal(out=var[:], in_=var[:])  # rsqrt(var+eps)
        nc.vector.tensor_tensor(out=ab[0:Cout, 0:1], in0=var[:],
                                in1=gbe[:, 0:1], op=OP.mult)       # a
        nc.vector.tensor_tensor(out=m2[:], in0=ab[0:Cout, 0:1], in1=mean[:],
                                op=OP.mult)
        nc.vector.tensor_tensor(out=ab[0:Cout, 1:2], in0=gbe[:, 1:2],
                                in1=m2[:], op=OP.subtract)         # b
        if dup:
            nc.vector.tensor_copy(out=ab[Cout:2 * Cout, :], in_=ab[0:Cout, :])


def _bn_allreduce(tc, li, sm, sq, gbe, ab, arin, arout, dup):
    """Reduce per-chunk stat slots, AllReduce across 8 cores, compute
    per-channel scale a = g*rsqrt(var+eps) and bias b = be - a*mean."""
    nc = tc.nc
    C = sm.shape[0]
    with tc.tile_pool(name=f"bn{li}", bufs=1) as bp:
        st = bp.tile([C, 2], F32)
        nc.vector.tensor_reduce(out=st[:, 0:1], in_=sm[:],
                                axis=mybir.AxisListType.X, op=OP.add)
        nc.vector.tensor_reduce(out=st[:, 1:2], in_=sq[:],
                                axis=mybir.AxisListType.X, op=OP.add)
        nc.sync.dma_start(out=arin[:], in_=st[:])
        if getattr(nc, "_single_core_nocoll", False):
            nc.sync.dma_start(out=arout[:], in_=arin[:])
        else:
            nc.gpsimd.collective_compute(
                "AllReduce", OP.add, replica_groups=[list(range(NCORES))],
                ins=[arin.opt()], outs=[arout.opt()])
        ar = bp.tile([C, 2], F32)
        nc.sync.dma_start(out=ar[:], in_=arout[:])
        mean = bp.tile([C, 1], F32)
        var = bp.tile([C, 1], F32)
        nc.vector.tensor_scalar_mul(mean[:], ar[:, 0:1], 1.0 / NTOT)
        nc.vector.tensor_scalar_mul(var[:], ar[:, 1:2], 1.0 / NTOT)
        m2 = bp.tile([C, 1], F32)
        nc.vector.tensor_tensor(out=m2[:], in0=mean[:], in1=mean[:], op=OP.mult)
        nc.vector.tensor_tensor(out=var[:], in0=var[:], in1=m2[:], op=OP.subtract)
        nc.vector.tensor_scalar_add(var[:], var[:], BN_EPS)
        nc.scalar.activation(out=var[:], in_=var[:], func=AF.Sqrt)
        nc.vector.reciprocal(out=var[:], in_=var[:])  # rsqrt(var+eps)
        nc.vector.tensor_tensor(out=ab[0:C, 0:1], in0=var[:], in1=gbe[:, 0:1],
                                op=OP.mult)            # a
        nc.vector.tensor_tensor(out=m2[:], in0=ab[0:C, 0:1], in1=mean[:],
                                op=OP.mult)
        nc.vector.tensor_tensor(out=ab[0:C, 1:2], in0=gbe[:, 1:2], in1=m2[:],
                                op=OP.subtract)        # b = be - a*mean
        if dup:
            nc.vector.tensor_copy(out=ab[C:2 * C, :], in_=ab[0:C, :])


_PROGRAM = None
LAST_RESULT = None


def _get_program():
    global _PROGRAM
    if _PROGRAM is None:
        _PROGRAM = _build_program()
    return _PROGRAM


def _prep_core_inputs(points1, points2, W1, W2, W3, gs, bes, b, h):
    p1 = points1[b]          # [3, N]
    p2 = points2[b]
    q = p1[:, h * QPC:(h + 1) * QPC]            # [3, QPC]
    qf = np.concatenate([2.0 * q, np.ones((1, QPC), np.float32)], axis=0)

    # rows 16g+{0..2}: coords (gather bands); row 3: -|c|^2 (dist matmul rhs
    # reads rows 0:4, and band-0 row 3 is never gathered)
    gtab = np.zeros((128, N), np.float32)
    gtab2 = np.zeros((128, N), np.float32)
    for g in range(8):
        gtab[16 * g + 0:16 * g + 3] = p1
        gtab2[16 * g + 0:16 * g + 3] = p2
    gtab[3] = -(p1 * p1).sum(axis=0)
    gtab2[3] = -(p2 * p2).sum(axis=0)
    qraw = np.zeros((4, QPC), np.float32)
    qraw[0:3] = q
    qsqv = (q * q).sum(axis=0).reshape(NT, 128).T.astype(np.float32)  # [128, NT]

    def dup128(w):      # [64, C] -> [128, C] duplicated
        return np.concatenate([w, w], axis=0).astype(np.float16)

    selw = np.zeros((8, 128), np.float32)
    for g in range(8):
        for c3 in range(3):
            selw[g, 16 * g + c3] = 1.0

    return {
        "selw": selw,
        "qf": qf.astype(np.float32),
        "gt": gtab, "gt2": gtab2,
        "qr": qraw, "qsq": np.ascontiguousarray(qsqv),
        "w1t": np.ascontiguousarray(W1.T).astype(np.float16),
        "w2t": dup128(np.ascontiguousarray(W2.T)),
        "w3t": dup128(np.ascontiguousarray(W3.T)),
        "gb1": np.stack([gs[0], bes[0]], axis=1).astype(np.float32),
        "gb2": np.stack([gs[1], bes[1]], axis=1).astype(np.float32),
        "gb3": np.stack([gs[2], bes[2]], axis=1).astype(np.float32),
        "w2f": np.ascontiguousarray(W2.T).astype(np.float32),
        "w3f": np.ascontiguousarray(W3.T).astype(np.float32),
    }


def kernel(points1, points2, k, t, W1, b1, g1, be1, W2, b2, g2, be2,
           W3, b3, g3, be3):
    # b1/b2/b3 cancel inside train-mode BatchNorm; t is unused by the net.
    assert int(np.asarray(k)) == KNN
    points1 = np.asarray(points1, np.float32)
    points2 = np.asarray(points2, np.float32)
    gs = [np.asarray(g1, np.float32), np.asarray(g2, np.float32),
          np.asarray(g3, np.float32)]
    bes = [np.asarray(be1, np.float32), np.asarray(be2, np.float32),
           np.asarray(be3, np.float32)]
    Ws = [np.asarray(W1, np.float32), np.asarray(W2, np.float32),
          np.asarray(W3, np.float32)]

    in_maps = []
    for c in range(NCORES):
        b, h = divmod(c, 2)
        in_maps.append(_prep_core_inputs(points1, points2, *Ws, gs, bes, b, h))

    nc = _get_program()
    bkr = run_bass_kernel_spmd(nc, in_maps, list(range(NCORES)))
    global LAST_RESULT
    LAST_RESULT = bkr
    res = bkr.results

    out = np.zeros((B, 3, N), np.float32)
    for c in range(NCORES):
        b, h = divmod(c, 2)
        out[b, :, h * QPC:(h + 1) * QPC] = res[c]["out"]
    return out


# revision 16
# speedup vs baseline: 1.4365x; 1.0043x over previous
"""PointsFusion Trainium2 kernel.

Pipeline per batch b (B=4, N=4096, k=32):
  knn1 = 32-NN of p1 in p1, knn2 = 32-NN of p1 in p2 (exact, via DVE 8-max rounds)
  gather neighbor coords, features (resi, dist) -> conv(4->64)->BN->relu
  -> conv(64->64)->BN->relu -> conv(64->128)->BN->relu -> channel-max scores
  -> softmax over 64 neighbors -> weighted sum of neighbor coords.

Sharding: 8 cores = (batch b, half h of the 4096 query points). BatchNorm uses
global batch stats -> 3 tiny AllReduces of per-channel sum/sumsq.

Layouts (per 128-query tile):
  pixel space: 16 chunks of 512; chunk c = kn*8+g, pixel j = c*512 + s*16 + p
  (g = query group, p = query-in-group, s = neighbor slot, kn = which knn).
  64-channel activations are packed [128, 4096]: chunk c lives at partitions
  64*(c%2)..+64, free 512*(c//2)..+512 (keeps matmul rhs bases in {0, 64}).

Self-contained: hardcodes shapes; no sibling imports.
"""

import sys

import numpy as np

for _p in ("/opt/trn_rl_repo", "/opt/pypackages"):
    if _p not in sys.path:
        sys.path.append(_p)

import concourse.bass as bass  # noqa: E402  (imported for side effects/typing)
import concourse.mybir as mybir  # noqa: E402
import concourse.tile as tile  # noqa: E402
from concourse import bacc, bass_isa  # noqa: E402
from concourse.bass_utils import run_bass_kernel_spmd  # noqa: E402
from concourse.masks import make_identity  # noqa: E402

F32 = mybir.dt.float32
F32R = mybir.dt.float32r
F16 = mybir.dt.float16
U16 = mybir.dt.uint16
I16 = mybir.dt.int16
AF = mybir.ActivationFunctionType
OP = mybir.AluOpType

NCORES = 8
B = 4
N = 4096          # candidate points per batch
KNN = 32          # neighbors per knn
QPC = 2048        # query points per core
NT = 16           # query tiles of 128 per core
C1, C2, C3 = 64, 64, 128
NTOT = float(B * N * 2 * KNN)   # BN stat count (global)
BN_EPS = 1e-3
NEG = -1.0e30


def _pk(cc):
    """packed [128, 4096] slice coords for chunk cc."""
    return 64 * (cc % 2), 512 * (cc // 2)


def _r(ap):
    return ap.bitcast(F32R)


def _build_program(single=False):
    nc = bacc.Bacc(
        "TRN2", target_bir_lowering=False, debug=False,
        num_devices=1 if single else NCORES,
    )
    nc._single_core_nocoll = single

    ap = {}
    def din(name, shape, dt=F32):
        ap[name] = nc.dram_tensor(name, shape, dt, kind="ExternalInput").ap()
    din("qf", [4, QPC])
    din("gt", [128, N])
    din("qr", [4, QPC])
    din("qsq", [128, NT])
    din("w1t", [4, C1], F16)
    din("w2t", [128, C2], F16)     # duplicated at partition 64
    din("w3t", [128, C3], F16)     # duplicated at partition 64
    din("gt2", [128, N])
    din("gb1", [C1, 2])
    din("gb2", [C2, 2])
    din("gb3", [C3, 2])
    din("selw", [8, 128])
    din("w2f", [C1, C2])
    din("w3f", [C2, C3])

    ap["out"] = nc.dram_tensor("out", [3, QPC], F32, kind="ExternalOutput").ap()

    ap["y1d"] = nc.dram_tensor("y1d", [NT, 128, 4096], F16).ap()
    ap["y2d"] = nc.dram_tensor("y2d", [NT, 128, 4096], F16).ap()
    ap["y3d"] = nc.dram_tensor("y3d", [NT, C3, 8192], F16).ap()
    ap["g1d"] = nc.dram_tensor("g1d", [NT, 128, 512], F32).ap()
    ap["g2d"] = nc.dram_tensor("g2d", [NT, 128, 512], F32).ap()
    for i, c in ((0, C1), (1, C2), (2, C3)):
        ap[f"arin{i}"] = nc.dram_tensor(f"arin{i}", [c * 2], F32).ap()
        ap[f"arout{i}"] = nc.dram_tensor(f"arout{i}", [c * 2], F32).ap()

    with tile.TileContext(nc) as tc:
        _kernel_body(tc, ap)
    nc.compile()
    return nc


def _kernel_body(tc, d):
    nc = tc.nc
    from contextlib import ExitStack

    ctx = ExitStack()
    with ctx:
        # constants alive through the whole kernel
        cpool = ctx.enter_context(tc.tile_pool(name="consts", bufs=1))
        qf = cpool.tile([4, QPC], F32)
        qr = cpool.tile([4, QPC], F32)
        qsq = cpool.tile([128, NT], F32)
        w1 = cpool.tile([4, C1], F16)
        w2 = cpool.tile([128, C2], F16)
        w3 = cpool.tile([128, C3], F16)
        gb1 = cpool.tile([C1, 2], F32)
        gb2 = cpool.tile([C2, 2], F32)
        gb3 = cpool.tile([C3, 2], F32)
        selw = cpool.tile([8, 128], F32)
        w2f = cpool.tile([C1, C2], F32)
        w3f = cpool.tile([C2, C3], F32)
        ident = cpool.tile([128, 128], F32)
        make_identity(nc, ident[:])
        for nm, sb in [("qf", qf), ("qr", qr), ("qsq", qsq), ("w1t", w1),
                       ("w2t", w2), ("w3t", w3), ("gb1", gb1), ("gb2", gb2),
                       ("gb3", gb3), ("selw", selw), ("w2f", w2f),
                       ("w3f", w3f)]:
            nc.sync.dma_start(out=sb[:], in_=d[nm][:])

        spool = ctx.enter_context(tc.tile_pool(name="stats", bufs=1))
        sm1 = spool.tile([C1, NT * 16], F32)
        sq1 = spool.tile([C1, NT * 16], F32)
        sxa2 = spool.tile([128, NT], F32)
        sqp2 = spool.tile([128, NT * 8], F32)
        sxa3 = spool.tile([128, NT], F32)
        sqp3 = spool.tile([C3, NT * 16], F32)
        ab1 = spool.tile([128, 2], F32)   # col0 = scale a, col1 = bias b (dup at 64)
        ab2 = spool.tile([128, 2], F32)
        ab3 = spool.tile([C3, 2], F32)
        # qball[16g+c, t*16+p] = q coord c of query (t, g, p)
        qball = spool.tile([128, NT * 16], F32)

        # ---------------- Phase 1: knn + gather + feat + conv1 ----------------
        with tc.tile_pool(name="p1knn", bufs=1) as kpool, \
             tc.tile_pool(name="p1m", bufs=6) as mpool, \
             tc.tile_pool(name="p1psum", bufs=2, space="PSUM") as pp, \
             tc.tile_pool(name="p1tp", bufs=2, space="PSUM") as tpp, \
             tc.tile_pool(name="p1cpsum", bufs=3, space="PSUM") as cp, \
             tc.tile_pool(name="p1feat", bufs=1) as fpool, \
             tc.tile_pool(name="p1kv", bufs=3) as kvp, \
             tc.tile_pool(name="p1work", bufs=2) as wp, \
             tc.tile_pool(name="p1y", bufs=2) as yp:
            gt = kpool.tile([128, N], F32)
            gt2 = kpool.tile([128, N], F32)
            for nm, sb in [("gt", gt), ("gt2", gt2)]:
                nc.sync.dma_start(out=sb[:], in_=d[nm][:])
            for cc in range(3):
                nc.sync.dma_start(
                    out=qball[cc::16, :].rearrange("g (t p) -> g t p", t=NT),
                    in_=d["qr"][cc:cc + 1, :].rearrange(
                        "c (t g p) -> (c g) t p", t=NT, g=8))

            def emit_knn(t):
                vals = kvp.tile([128, 64], F32, tag="vals")
                idxu = kvp.tile([128, 64], U16, tag="idxu")
                idxi = kvp.tile([128, 64], I16, tag="idxi")
                for kn, tab in ((0, gt), (1, gt2)):
                    msb = mpool.tile([128, N], F32, tag="msb")
                    # M = 2 q.c - |c|^2 (maximize == nearest)
                    for ch in range(8):
                        pm = pp.tile([128, 512], F32, tag="pm")
                        nc.tensor.matmul(
                            out=pm[:],
                            lhsT=qf[:, t * 128:(t + 1) * 128],
                            rhs=tab[0:4, ch * 512:(ch + 1) * 512],
                            start=True, stop=True,
                        )
                        nc.scalar.activation(
                            out=msb[:, ch * 512:(ch + 1) * 512], in_=pm[:],
                            func=AF.Identity)
                    # top-32 rounds
                    for r in range(4):
                        v8 = vals[:, kn * 32 + r * 8: kn * 32 + r * 8 + 8]
                        i8 = idxu[:, kn * 32 + r * 8: kn * 32 + r * 8 + 8]
                        nc.vector.max(out=v8, in_=msb[:])
                        nc.vector.max_index(out=i8, in_max=v8, in_values=msb[:])
                        if r < 3:
                            nc.vector.match_replace(
                                out=msb[:], in_to_replace=v8,
                                in_values=msb[:], imm_value=NEG)
                nc.vector.tensor_copy(out=idxi[:], in_=idxu[:])
                return vals, idxi

            def emit_post(t, vals, idxi):
                # gather neighbor coords; both tables carry xyz on band rows
                # 16g+{0..2} (gt = p1 for knn1, gt2 = p2 for knn2); spill raw
                # for the fusion phase
                g1 = wp.tile([128, 512], F32, tag="g1")
                g2 = wp.tile([128, 512], F32, tag="g2")
                nc.gpsimd.ap_gather(
                    out_ap=g1[:], in_ap=gt[:], idxs_ap=idxi[:, 0:32],
                    channels=128, num_elems=N, d=1, num_idxs=512)
                nc.gpsimd.ap_gather(
                    out_ap=g2[:], in_ap=gt2[:], idxs_ap=idxi[:, 32:64],
                    channels=128, num_elems=N, d=1, num_idxs=512)
                nc.sync.dma_start(out=d["g1d"][t], in_=g1[:])
                nc.sync.dma_start(out=d["g2d"][t], in_=g2[:])

                # resi = nn - q, in band layout (out of place to avoid WAR
                # with the raw spill)
                qb = qball[:, t * 16:(t + 1) * 16]
                g1r = wp.tile([128, 512], F16, tag="g1r")
                g2r = wp.tile([128, 512], F16, tag="g2r")
                for gsrc, gdst in ((g1, g1r), (g2, g2r)):
                    nc.vector.tensor_tensor(
                        out=gdst[:].rearrange("c (s p) -> c s p", s=32),
                        in0=gsrc[:].rearrange("c (s p) -> c s p", s=32),
                        in1=qb.unsqueeze(1).to_broadcast([128, 32, 16]),
                        op=OP.subtract)

                # conv1 rhs must start at partition 0: strided-partition DMAs
                # into a flat [4, 8192] tile (3 per table)
                feat = fpool.tile([4, 8192], F16, tag="feat")
                for kn, gsrc in ((0, g1r), (1, g2r)):
                    for cc in range(3):
                        nc.sync.dma_start(
                            out=feat[cc:cc + 1, kn * 4096:(kn + 1) * 4096]
                                .rearrange("c (g sp) -> c g sp", g=8),
                            in_=gsrc[cc::16, :])

                # dist = sqrt(relu(|q|^2 - val)) into feat row 3
                d2 = wp.tile([128, 64], F32, tag="d2")
                nc.scalar.activation(
                    out=d2[:], in_=vals[:], func=AF.Relu,
                    scale=-1.0, bias=qsq[:, t:t + 1])
                nc.scalar.activation(out=d2[:], in_=d2[:], func=AF.Sqrt)
                # shuffle dist to pixel layout: PE-transpose to [nbr, query],
                # then one DMA per knn half
                dtp = tpp.tile([64, 128], F32, tag="dtp")
                nc.tensor.transpose(out=dtp[:], in_=d2[:], identity=ident[:])
                d2t = wp.tile([64, 128], F16, tag="d2t")
                nc.scalar.activation(out=d2t[:], in_=dtp[:], func=AF.Identity)
                for kn in (0, 1):
                    for g in range(8):
                        c = kn * 8 + g
                        nc.sync.dma_start(
                            out=feat[3:4, c * 512:(c + 1) * 512]
                                .rearrange("c (s p) -> c s p", s=32),
                            in_=d2t[kn * 32:(kn + 1) * 32,
                                    16 * g:16 * g + 16])

                # conv1: 16 chunks -> y1 packed [128, 4096]
                y1 = yp.tile([128, 4096], F16, tag="y1")
                for c in range(16):
                    bp_, fo = _pk(c)
                    pc = cp.tile([C1, 512], F32, tag="pc1")
                    nc.tensor.matmul(
                        out=pc[:],
                        lhsT=w1[:],
                        rhs=feat[:, c * 512:(c + 1) * 512],
                        start=True, stop=True)
                    nc.scalar.activation(
                        out=y1[bp_:bp_ + 64, fo:fo + 512], in_=pc[:],
                        func=AF.Identity,
                        accum_out=sm1[:, t * 16 + c: t * 16 + c + 1])
                    sqs = wp.tile([C1, 512], F32, tag="sqs")
                    nc.scalar.activation(
                        out=sqs[:], in_=pc[:], func=AF.Square,
                        accum_out=sq1[:, t * 16 + c: t * 16 + c + 1])
                nc.sync.dma_start(out=d["y1d"][t], in_=y1[:])

            # software pipeline with 2-tile lookahead: the Scalar queue sees
            # msb copies for tiles t+1/t+2 before tile t's post-knn Scalar
            # work, so the DVE top-k stream always has buffered input.
            import collections
            pend = collections.deque()
            pend.append(emit_knn(0))
            pend.append(emit_knn(1))
            for t in range(NT):
                if t + 2 < NT:
                    pend.append(emit_knn(t + 2))
                emit_post(t, *pend.popleft())

        _bn_allreduce(tc, 0, sm1, sq1, gb1, ab1, d["arin0"], d["arout0"], True)

        # ---------------- Phase 2: apply BN1+relu, conv2 ----------------
        with tc.tile_pool(name="p2y", bufs=2) as yp, \
             tc.tile_pool(name="p2psum", bufs=4, space="PSUM") as cp, \
             tc.tile_pool(name="p2work", bufs=2) as wp:
            for t in range(NT):
                y1 = yp.tile([128, 4096], F16, tag="y1l")
                nc.sync.dma_start(out=y1[:], in_=d["y1d"][t])
                nc.scalar.activation(
                    out=y1[:], in_=y1[:], func=AF.Relu,
                    scale=ab1[:, 0:1], bias=ab1[:, 1:2],
                    accum_out=sxa2[:, t:t + 1])
                y2 = yp.tile([128, 4096], F16, tag="y2")
                for c in range(16):
                    bp_, fo = _pk(c)
                    pc = cp.tile([C2, 512], F32, tag="pc2")
                    nc.tensor.matmul(
                        out=pc[:], lhsT=w2[bp_:bp_ + 64, :],
                        rhs=y1[bp_:bp_ + 64, fo:fo + 512],
                        start=True, stop=True)
                    nc.scalar.activation(
                        out=y2[bp_:bp_ + 64, fo:fo + 512], in_=pc[:],
                        func=AF.Identity)
                nc.sync.dma_start(out=d["y2d"][t], in_=y2[:])
                ysq = wp.tile([128, 4096], F16, tag="ysq2")
                nc.vector.tensor_tensor(out=ysq[:], in0=y2[:], in1=y2[:],
                                        op=OP.mult)
                nc.vector.tensor_reduce(
                    out=sqp2[:, t * 8:(t + 1) * 8],
                    in_=ysq[:].rearrange("c (h f) -> c h f", h=8),
                    axis=mybir.AxisListType.X, op=OP.add)

        _bn_finalize23(tc, 1, sxa2, sqp2, w2f, gb2, ab2, d["arin1"],
                       d["arout1"], dup=True)

        # ---------------- Phase 3: apply BN2+relu, conv3 ----------------
        with tc.tile_pool(name="p3y", bufs=2) as yp, \
             tc.tile_pool(name="p3psum", bufs=4, space="PSUM") as cp, \
             tc.tile_pool(name="p3work", bufs=2) as wp:
            for t in range(NT):
                y2 = yp.tile([128, 4096], F16, tag="y2l")
                nc.sync.dma_start(out=y2[:], in_=d["y2d"][t])
                nc.scalar.activation(
                    out=y2[:], in_=y2[:], func=AF.Relu,
                    scale=ab2[:, 0:1], bias=ab2[:, 1:2],
                    accum_out=sxa3[:, t:t + 1])
                y3 = yp.tile([C3, 8192], F16, tag="y3")
                for c in range(16):
                    bp_, fo = _pk(c)
                    pc = cp.tile([C3, 512], F32, tag="pc3")
                    nc.tensor.matmul(
                        out=pc[:], lhsT=w3[bp_:bp_ + 64, :],
                        rhs=y2[bp_:bp_ + 64, fo:fo + 512],
                        start=True, stop=True)
                    nc.scalar.activation(
                        out=y3[:, c * 512:(c + 1) * 512], in_=pc[:],
                        func=AF.Identity)
                nc.sync.dma_start(out=d["y3d"][t], in_=y3[:])
                ysq = wp.tile([C3, 8192], F16, tag="ysq3")
                nc.vector.tensor_tensor(out=ysq[:], in0=y3[:], in1=y3[:],
                                        op=OP.mult)
                nc.vector.tensor_reduce(
                    out=sqp3[:, t * 16:(t + 1) * 16],
                    in_=ysq[:].rearrange("c (h f) -> c h f", h=16),
                    axis=mybir.AxisListType.X, op=OP.add)

        _bn_finalize23(tc, 2, sxa3, sqp3, w3f, gb3, ab3, d["arin2"],
                       d["arout2"], dup=False)

        # ------------- Phase 4: scores, softmax, fusion, output -------------
        with tc.tile_pool(name="p4y", bufs=2) as yp, \
             tc.tile_pool(name="p4sc", bufs=1) as scp, \
             tc.tile_pool(name="p4work", bufs=2) as wp, \
             tc.tile_pool(name="p4psum", bufs=2, space="PSUM") as pp4, \
             tc.tile_pool(name="p4out", bufs=1) as op_:
            outsb = op_.tile([4, QPC], F32)
            pscore = scp.tile([128, 8192], F32)
            for t in range(NT):
                y3 = yp.tile([C3, 8192], F16, tag="y3l")
                nc.sync.dma_start(out=y3[:], in_=d["y3d"][t])
                y3r = yp.tile([C3, 8192], F32, tag="y3r")
                nc.scalar.activation(
                    out=y3r[:], in_=y3[:], func=AF.Relu,
                    scale=ab3[:, 0:1], bias=ab3[:, 1:2])
                # channel-max scores via gpsimd partition reduce; batch the
                # row-0 extraction into one DMA per knn half
                for c in range(16):
                    nc.gpsimd.partition_all_reduce(
                        out_ap=pscore[:, c * 512:(c + 1) * 512],
                        in_ap=y3r[:, c * 512:(c + 1) * 512],
                        channels=128, reduce_op=bass_isa.ReduceOp.max)
                scA = wp.tile([8, 512], F32, tag="scA")
                scB = wp.tile([8, 512], F32, tag="scB")
                for kn, sct in ((0, scA), (1, scB)):
                    nc.sync.dma_start(
                        out=sct[:],
                        in_=pscore[0:1, kn * 4096:(kn + 1) * 4096]
                            .rearrange("c (g sp) -> c g sp", g=8))
                # softmax over the 64 neighbors of each query
                qmA = wp.tile([8, 16], F32, tag="qmA")
                qmB = wp.tile([8, 16], F32, tag="qmB")
                for sct, qm in ((scA, qmA), (scB, qmB)):
                    nc.vector.tensor_reduce(
                        out=qm[:],
                        in_=sct[:].rearrange("c (s p) -> c p s", s=32),
                        axis=mybir.AxisListType.X, op=OP.max)
                nc.vector.tensor_tensor(
                    out=qmA[:], in0=qmA[:], in1=qmB[:], op=OP.max)
                exA = wp.tile([8, 512], F32, tag="exA")
                exB = wp.tile([8, 512], F32, tag="exB")
                for sct, ext in ((scA, exA), (scB, exB)):
                    nc.vector.tensor_tensor(
                        out=ext[:].rearrange("c (s p) -> c s p", s=32),
                        in0=sct[:].rearrange("c (s p) -> c s p", s=32),
                        in1=qmA[:].unsqueeze(1).to_broadcast([8, 32, 16]),
                        op=OP.subtract)
                    nc.scalar.activation(out=ext[:], in_=ext[:], func=AF.Exp)
                esA = wp.tile([8, 16], F32, tag="esA")
                esB = wp.tile([8, 16], F32, tag="esB")
                for ext, est in ((exA, esA), (exB, esB)):
                    nc.vector.tensor_reduce(
                        out=est[:],
                        in_=ext[:].rearrange("c (s p) -> c p s", s=32),
                        axis=mybir.AxisListType.X, op=OP.add)
                nc.vector.tensor_tensor(
                    out=esA[:], in0=esA[:], in1=esB[:], op=OP.add)
                nc.vector.reciprocal(out=esA[:], in_=esA[:])
                for ext in (exA, exB):
                    nc.vector.tensor_tensor(
                        out=ext[:].rearrange("c (s p) -> c s p", s=32),
                        in0=ext[:].rearrange("c (s p) -> c s p", s=32),
                        in1=esA[:].unsqueeze(1).to_broadcast([8, 32, 16]),
                        op=OP.mult)
                # fusion: replicate weight rows onto band partitions via a
                # selector matmul, multiply with raw coords, segment-reduce
                g1 = wp.tile([128, 512], F32, tag="g1l")
                g2 = wp.tile([128, 512], F32, tag="g2l")
                nc.sync.dma_start(out=g1[:], in_=d["g1d"][t])
                nc.sync.dma_start(out=g2[:], in_=d["g2d"][t])
                wr1 = wp.tile([128, 512], F32, tag="wr1")
                wr2 = wp.tile([128, 512], F32, tag="wr2")
                for ext, wr in ((exA, wr1), (exB, wr2)):
                    pw = pp4.tile([128, 512], F32, tag="pw")
                    nc.tensor.matmul(
                        out=pw[:], lhsT=selw[:],
                        rhs=ext[:], start=True, stop=True)
                    nc.scalar.activation(out=wr[:], in_=pw[:], func=AF.Identity)
                pr = wp.tile([128, 512], F32, tag="pr")
                nc.vector.tensor_tensor(out=pr[:], in0=g1[:], in1=wr1[:],
                                        op=OP.mult)
                nc.vector.tensor_tensor(out=wr2[:], in0=g2[:], in1=wr2[:],
                                        op=OP.mult)
                nc.vector.tensor_tensor(out=pr[:], in0=pr[:], in1=wr2[:],
                                        op=OP.add)
                fp = wp.tile([128, 16], F32, tag="fp")
                nc.vector.tensor_reduce(
                    out=fp[:], in_=pr[:].rearrange("c (s p) -> c p s", s=32),
                    axis=mybir.AxisListType.X, op=OP.add)
                for cc in range(3):
                    nc.sync.dma_start(
                        out=outsb[cc:cc + 1, t * 128:(t + 1) * 128]
                            .rearrange("c (g p) -> c g p", g=8),
                        in_=fp[cc::16, :])
            nc.sync.dma_start(out=d["out"][:], in_=outsb[0:3, :])


def _bn_finalize23(tc, li, sxa, sqp, wf, gbe, ab, arin, arout, dup):
    """BN stats for conv2/conv3: sum(y) = W @ sum(x) (sum(x) from the relu
    pass accums), sum(y^2) from the DVE per-tile partials. AllReduce packs
    [sum(x) | sum(y^2)] as one [C, 2] tile."""
    nc = tc.nc
    Cin = 64
    Cout = wf.shape[1]
    with tc.tile_pool(name=f"bnf{li}", bufs=1) as bp, \
         tc.tile_pool(name=f"bnfp{li}", bufs=1, space="PSUM") as pp:
        st = bp.tile([Cout, 2], F32)
        if Cout > Cin:
            nc.vector.tensor_scalar_mul(st[:, 0:1], st[:, 0:1], 0.0)
        red = bp.tile([128, 2], F32)
        hi = bp.tile([64, 2], F32)
        nc.vector.tensor_reduce(out=red[:, 0:1], in_=sxa[:],
                                axis=mybir.AxisListType.X, op=OP.add)
        nc.vector.tensor_reduce(out=red[0:sqp.shape[0], 1:2], in_=sqp[:],
                                axis=mybir.AxisListType.X, op=OP.add)
        # fold the packed halves (DVE needs equal partition bases -> bounce
        # the upper half through a base-0 tile)
        nc.vector.tensor_copy(out=hi[:], in_=red[64:128, :])
        nc.vector.tensor_tensor(out=st[0:Cin, 0:1], in0=red[0:64, 0:1],
                                in1=hi[:, 0:1], op=OP.add)
        if Cout == 64:   # packed couts: fold halves
            nc.vector.tensor_tensor(out=st[:, 1:2], in0=red[0:64, 1:2],
                                    in1=hi[:, 1:2], op=OP.add)
        else:
            nc.vector.tensor_copy(out=st[:, 1:2], in_=red[:, 1:2])
        nc.sync.dma_start(out=arin[:], in_=st[:])
        if getattr(nc, "_single_core_nocoll", False):
            nc.sync.dma_start(out=arout[:], in_=arin[:])
        else:
            nc.gpsimd.collective_compute(
                "AllReduce", OP.add, replica_groups=[list(range(NCORES))],
                ins=[arin.opt()], outs=[arout.opt()])
        ar = bp.tile([Cout, 2], F32)
        nc.sync.dma_start(out=ar[:], in_=arout[:])
        # sum(y) = W @ sum(x): lhsT = W^T [Cin, Cout]
        ps = pp.tile([Cout, 1], F32)
        nc.tensor.matmul(out=ps[:], lhsT=wf[:], rhs=ar[0:Cin, 0:1],
                         start=True, stop=True)
        mean = bp.tile([Cout, 1], F32)
        nc.scalar.activation(out=mean[:], in_=ps[:], func=AF.Copy,
                             scale=1.0 / NTOT)
        var = bp.tile([Cout, 1], F32)
        nc.vector.tensor_scalar_mul(var[:], ar[:, 1:2], 1.0 / NTOT)
        m2 = bp.tile([Cout, 1], F32)
        nc.vector.tensor_tensor(out=m2[:], in0=mean[:], in1=mean[:], op=OP.mult)
        nc.vector.tensor_tensor(out=var[:], in0=var[:], in1=m2[:],
                                op=OP.subtract)
        nc.vector.tensor_scalar_add(var[:], var[:], BN_EPS)
        nc.scalar.activation(out=var[:], in_=var[:], func=AF.Sqrt)
        nc.vector.reciprocal(out=var[:], in_=var[:])  # rsqrt(var+eps)
        nc.vector.tensor_tensor(out=ab[0:Cout, 0:1], in0=var[:],
                                in1=gbe[:, 0:1], op=OP.mult)       # a
        nc.vector.tensor_tensor(out=m2[:], in0=ab[0:Cout, 0:1], in1=mean[:],
                                op=OP.mult)
        nc.vector.tensor_tensor(out=ab[0:Cout, 1:2], in0=gbe[:, 1:2],
                                in1=m2[:], op=OP.subtract)         # b
        if dup:
            nc.vector.tensor_copy(out=ab[Cout:2 * Cout, :], in_=ab[0:Cout, :])


def _bn_allreduce(tc, li, sm, sq, gbe, ab, arin, arout, dup):
    """Reduce per-chunk stat slots, AllReduce across 8 cores, compute
    per-channel scale a = g*rsqrt(var+eps) and bias b = be - a*mean."""
    nc = tc.nc
    C = sm.shape[0]
    with tc.tile_pool(name=f"bn{li}", bufs=1) as bp:
        st = bp.tile([C, 2], F32)
        nc.vector.tensor_reduce(out=st[:, 0:1], in_=sm[:],
                                axis=mybir.AxisListType.X, op=OP.add)
        nc.vector.tensor_reduce(out=st[:, 1:2], in_=sq[:],
                                axis=mybir.AxisListType.X, op=OP.add)
        nc.sync.dma_start(out=arin[:], in_=st[:])
        if getattr(nc, "_single_core_nocoll", False):
            nc.sync.dma_start(out=arout[:], in_=arin[:])
        else:
            nc.gpsimd.collective_compute(
                "AllReduce", OP.add, replica_groups=[list(range(NCORES))],
                ins=[arin.opt()], outs=[arout.opt()])
        ar = bp.tile([C, 2], F32)
        nc.sync.dma_start(out=ar[:], in_=arout[:])
        mean = bp.tile([C, 1], F32)
        var = bp.tile([C, 1], F32)
        nc.vector.tensor_scalar_mul(mean[:], ar[:, 0:1], 1.0 / NTOT)
        nc.vector.tensor_scalar_mul(var[:], ar[:, 1:2], 1.0 / NTOT)
        m2 = bp.tile([C, 1], F32)
        nc.vector.tensor_tensor(out=m2[:], in0=mean[:], in1=mean[:], op=OP.mult)
        nc.vector.tensor_tensor(out=var[:], in0=var[:], in1=m2[:], op=OP.subtract)
        nc.vector.tensor_scalar_add(var[:], var[:], BN_EPS)
        nc.scalar.activation(out=var[:], in_=var[:], func=AF.Sqrt)
        nc.vector.reciprocal(out=var[:], in_=var[:])  # rsqrt(var+eps)
        nc.vector.tensor_tensor(out=ab[0:C, 0:1], in0=var[:], in1=gbe[:, 0:1],
                                op=OP.mult)            # a
        nc.vector.tensor_tensor(out=m2[:], in0=ab[0:C, 0:1], in1=mean[:],
                                op=OP.mult)
        nc.vector.tensor_tensor(out=ab[0:C, 1:2], in0=gbe[:, 1:2], in1=m2[:],
                                op=OP.subtract)        # b = be - a*mean
        if dup:
            nc.vector.tensor_copy(out=ab[C:2 * C, :], in_=ab[0:C, :])


_PROGRAM = None
LAST_RESULT = None


def _get_program():
    global _PROGRAM
    if _PROGRAM is None:
        _PROGRAM = _build_program()
    return _PROGRAM


def _prep_core_inputs(points1, points2, W1, W2, W3, gs, bes, b, h):
    p1 = points1[b]          # [3, N]
    p2 = points2[b]
    q = p1[:, h * QPC:(h + 1) * QPC]            # [3, QPC]
    qf = np.concatenate([2.0 * q, np.ones((1, QPC), np.float32)], axis=0)

    # rows 16g+{0..2}: coords (gather bands); row 3: -|c|^2 (dist matmul rhs
    # reads rows 0:4, and band-0 row 3 is never gathered)
    gtab = np.zeros((128, N), np.float32)
    gtab2 = np.zeros((128, N), np.float32)
    for g in range(8):
        gtab[16 * g + 0:16 * g + 3] = p1
        gtab2[16 * g + 0:16 * g + 3] = p2
    gtab[3] = -(p1 * p1).sum(axis=0)
    gtab2[3] = -(p2 * p2).sum(axis=0)
    qraw = np.zeros((4, QPC), np.float32)
    qraw[0:3] = q
    qsqv = (q * q).sum(axis=0).reshape(NT, 128).T.astype(np.float32)  # [128, NT]

    def dup128(w):      # [64, C] -> [128, C] duplicated
        return np.concatenate([w, w], axis=0).astype(np.float16)

    selw = np.zeros((8, 128), np.float32)
    for g in range(8):
        for c3 in range(3):
            selw[g, 16 * g + c3] = 1.0

    return {
        "selw": selw,
        "qf": qf.astype(np.float32),
        "gt": gtab, "gt2": gtab2,
        "qr": qraw, "qsq": np.ascontiguousarray(qsqv),
        "w1t": np.ascontiguousarray(W1.T).astype(np.float16),
        "w2t": dup128(np.ascontiguousarray(W2.T)),
        "w3t": dup128(np.ascontiguousarray(W3.T)),
        "gb1": np.stack([gs[0], bes[0]], axis=1).astype(np.float32),
        "gb2": np.stack([gs[1], bes[1]], axis=1).astype(np.float32),
        "gb3": np.stack([gs[2], bes[2]], axis=1).astype(np.float32),
        "w2f": np.ascontiguousarray(W2.T).astype(np.float32),
        "w3f": np.ascontiguousarray(W3.T).astype(np.float32),
    }


def kernel(points1, points2, k, t, W1, b1, g1, be1, W2, b2, g2, be2,
           W3, b3, g3, be3):
    # b1/b2/b3 cancel inside train-mode BatchNorm; t is unused by the net.
    assert int(np.asarray(k)) == KNN
    points1 = np.asarray(points1, np.float32)
    points2 = np.asarray(points2, np.float32)
    gs = [np.asarray(g1, np.float32), np.asarray(g2, np.float32),
          np.asarray(g3, np.float32)]
    bes = [np.asarray(be1, np.float32), np.asarray(be2, np.float32),
           np.asarray(be3, np.float32)]
    Ws = [np.asarray(W1, np.float32), np.asarray(W2, np.float32),
          np.asarray(W3, np.float32)]

    in_maps = []
    for c in range(NCORES):
        b, h = divmod(c, 2)
        in_maps.append(_prep_core_inputs(points1, points2, *Ws, gs, bes, b, h))

    nc = _get_program()
    bkr = run_bass_kernel_spmd(nc, in_maps, list(range(NCORES)))
    global LAST_RESULT
    LAST_RESULT = bkr
    res = bkr.results

    out = np.zeros((B, 3, N), np.float32)
    for c in range(NCORES):
        b, h = divmod(c, 2)
        out[b, :, h * QPC:(h + 1) * QPC] = res[c]["out"]
    return out
